# revision 1
# baseline (speedup 1.0000x reference)
"""Trainium2 Bass kernel for GNN mean aggregation (nn_AggrGSMean).

Computes, for t in {0,1}:
    out_t[b, v, :] = segment_sum(features_t over edges with dest v) / degree[b, v, t]
where degree[b, v, t] = max(count(adjacency[b, v, t, :] >= 0), 1).

Strategy (graph-partition sharding per the problem's sharding hint):
- Host: partition edges by destination-vertex range across 8 cores, sort each
  core's edges by destination, group into 128-vertex blocks.  Each block's edge
  list is padded to a whole number of 128-edge tiles.  Blocks are assigned to
  "slots" in decreasing-tile-count order so one static per-slot tile profile
  (max over cores/tables at each rank) serves all cores with ~8% less padding
  than a uniform max.  Features ship as bf16 hi+lo halves (their sum is the
  fp32 value to ~1e-5) plus the destination slot-in-block encoded as a float.
- Device (per core): for each slot, stream 128-edge tiles [hi64|lo64|negv]
  bf16; build a one-hot [128 edges x 128 vslots] in bf16 (iota == vslot) on
  DVE (a fraction on ScalarE via relu(1-(iota-v)^2)); one matmul per tile
  accumulates onehot.T @ [hi|lo] into PSUM [128, 128].  Degree comes from the
  adjacency slice on-chip; the hi/lo halves are summed by a strided
  tensor_reduce and the mean division rides the ScalarE copy (per-partition
  scale = 1/degree).
"""

import sys

if "/opt/trn_rl_repo" not in sys.path:
    sys.path.insert(0, "/opt/trn_rl_repo")

import ml_dtypes
import numpy as np

# Problem constants (hardcoded per contract)
B, V, T, N, F, M = 1, 100000, 2, 32, 64, 1600000
NCORES = 8
BLK = 128           # edges per tile (matmul contraction)
BLK_V = 96          # vertices per block / one-hot width
EW = 2 * F + 2      # bf16 words per edge row: 64 hi | 64 lo | negv f32 (2)
ADJ_G = 7

ONE_F32_U16 = np.array([0x0000, 0x3F80], dtype=np.uint16)  # f32 1.0 as 2 LE u16


class Cfg:
    def __init__(self, v=V, ncores=NCORES):
        self.V = v
        self.NCORES = ncores
        self.VLOC = v // ncores
        nblk = (self.VLOC + BLK_V - 1) // BLK_V
        self.NBLK = ((nblk + ADJ_G - 1) // ADJ_G) * ADJ_G
        self.VPAD = self.NBLK * BLK_V


_DEFAULT_CFG = Cfg()
_NC_CACHE = {}


def build_device_program(
    profile,
    cfg=_DEFAULT_CFG,
    act_frac=0.16,
    gp_frac=0.0,
    dve_chunk=16,
    gp_chunk=16,
):
    """Build + compile the per-core Bass program.

    profile: per-slot tile counts (len NBLK); same static schedule on all cores.
    One-hot builds are batched (k tiles per instruction via broadcast
    tensor_tensor is_equal) and split across DVE / GPSIMD / ScalarE by the
    given fractions to balance engine load.
    """
    from contextlib import ExitStack

    import concourse.tile as tile
    from concourse import bacc, mybir

    f32 = mybir.dt.float32
    bf16 = mybir.dt.bfloat16
    i32 = mybir.dt.int32
    NBLK = cfg.NBLK
    assert len(profile) == NBLK and NBLK % ADJ_G == 0
    t_max = max(profile)
    slot_elems = [BLK * ts * EW for ts in profile]  # edge rows are 128/tile
    slot_base = np.concatenate([[0], np.cumsum(slot_elems)]).astype(np.int64)
    total_elems = int(slot_base[-1])

    nc = bacc.Bacc("TRN2", target_bir_lowering=False, debug=False)
    feat_d = [
        nc.dram_tensor(f"feat{t}", [total_elems], bf16, kind="ExternalInput").ap()
        for t in range(T)
    ]
    adj_d = nc.dram_tensor(
        "adj", [NBLK // ADJ_G, BLK_V, ADJ_G * T * N], i32, kind="ExternalInput"
    ).ap()
    # iota_neg[e, j] = -j (f32) for DVE is_equal against negv;
    # iota_pos[e, j] = +j (bf16) for the ScalarE (j + negv)^2 path
    iota_n_d = nc.dram_tensor("iota_neg", [BLK, BLK_V], f32, kind="ExternalInput").ap()
    iota_p_d = nc.dram_tensor("iota_pos", [BLK, BLK_V], bf16, kind="ExternalInput").ap()
    out_d = nc.dram_tensor("out", [NBLK, BLK_V, T * F], f32, kind="ExternalOutput").ap()

    with tile.TileContext(nc) as tc, ExitStack() as ctx:
        const = ctx.enter_context(tc.tile_pool(name="const", bufs=1))
        featp = ctx.enter_context(tc.tile_pool(name="featp", bufs=6))
        adjp = ctx.enter_context(tc.tile_pool(name="adjp", bufs=3))
        degp = ctx.enter_context(tc.tile_pool(name="degp", bufs=3))
        ohdp = ctx.enter_context(tc.tile_pool(name="ohdp", bufs=6))
        ohgp = ctx.enter_context(tc.tile_pool(name="ohgp", bufs=3))
        ohap = ctx.enter_context(tc.tile_pool(name="ohap", bufs=7))
        redp = ctx.enter_context(tc.tile_pool(name="redp", bufs=5))
        outp = ctx.enter_context(tc.tile_pool(name="outp", bufs=4))
        psump = ctx.enter_context(tc.tile_pool(name="psum", bufs=6, space="PSUM"))

        iota_n = const.tile([BLK, BLK_V], f32)
        nc.sync.dma_start(out=iota_n[:], in_=iota_n_d[:])
        iota_p = const.tile([BLK, BLK_V], bf16)
        nc.sync.dma_start(out=iota_p[:], in_=iota_p_d[:])

        # --- one-hot build planning: weighted chunk assignment across engines ---
        n_tot = [0]
        n_act = [0]
        n_gp = [0]

        def plan_chunks(t_s):
            chunks = []
            i = 0
            while i < t_s:
                rest = t_s - i
                if n_gp[0] + gp_chunk <= gp_frac * (n_tot[0] + gp_chunk) and rest >= gp_chunk:
                    k = gp_chunk
                    chunks.append(("gp", i, k))
                    n_gp[0] += k
                elif n_act[0] < act_frac * n_tot[0]:
                    k = 1
                    chunks.append(("act", i, k))
                    n_act[0] += k
                else:
                    k = min(dve_chunk, rest)
                    chunks.append(("dve", i, k))
                i += k
                n_tot[0] += k
            return chunks

        def build_onehots(feat3, t_s):
            """Build all t_s one-hots for a slot; returns per-tile (tile, col0)."""
            refs = [None] * t_s
            for kind, i0, k in plan_chunks(t_s):
                if kind == "act":
                    negv = feat3[:, i0, 2 * F : 2 * F + 2].bitcast(f32)
                    y = ohap.tile([BLK, BLK_V], bf16, tag="y")
                    nc.scalar.activation(
                        y[:], iota_p[:], mybir.ActivationFunctionType.Square,
                        bias=negv, scale=1.0,
                    )
                    oh = ohap.tile([BLK, BLK_V], bf16, tag="oha")
                    nc.scalar.activation(
                        oh[:], y[:], mybir.ActivationFunctionType.Relu,
                        bias=1.0, scale=-1.0,
                    )
                    refs[i0] = (oh, 0)
                else:
                    eng = nc.gpsimd if kind == "gp" else nc.vector
                    pool_k = gp_chunk if kind == "gp" else dve_chunk
                    pool = ohgp if kind == "gp" else ohdp
                    oh = pool.tile([BLK, pool_k * BLK_V], bf16, tag="oh" + kind)
                    oh3 = oh[:, : k * BLK_V].rearrange("p (i v) -> p i v", v=BLK_V)
                    in0 = iota_n[:].unsqueeze(1).broadcast_to([BLK, k, BLK_V])
                    negv = feat3[:, i0 : i0 + k, 2 * F : 2 * F + 2].bitcast(f32)
                    in1 = negv.broadcast_to([BLK, k, BLK_V])
                    eng.tensor_tensor(oh3, in0, in1, op=mybir.AluOpType.is_equal)
                    for j in range(k):
                        refs[i0 + j] = (oh, j * BLK_V)
            return refs

        for bg in range(NBLK // ADJ_G):
            adj_t = adjp.tile([BLK_V, ADJ_G * T * N], i32)
            nc.sync.dma_start(out=adj_t[:], in_=adj_d[bg])
            val = degp.tile([BLK_V, ADJ_G * T * N], f32, tag="val")
            nc.vector.tensor_scalar(
                val[:], adj_t[:], 0, None, op0=mybir.AluOpType.is_ge
            )
            deg = degp.tile([BLK_V, ADJ_G * T], f32, tag="deg")
            nc.vector.tensor_reduce(
                deg[:],
                val[:].rearrange("p (g n) -> p g n", n=N),
                axis=mybir.AxisListType.X,
                op=mybir.AluOpType.add,
            )
            rec = degp.tile([BLK_V, ADJ_G * T], f32, tag="rec")
            nc.vector.tensor_scalar(
                deg[:], deg[:], 1.0, None, op0=mybir.AluOpType.max
            )
            nc.vector.reciprocal(rec[:], deg[:])

            for bo in range(ADJ_G):
                s = bg * ADJ_G + bo
                t_s = profile[s]
                out_t = outp.tile([BLK_V, T * F], f32)
                for t in range(T):
                    feat_t = featp.tile([BLK, t_max * EW], bf16, tag="feat")
                    src = feat_d[t][
                        int(slot_base[s]) : int(slot_base[s + 1])
                    ].rearrange("(e w) -> e w", w=t_s * EW)
                    nc.sync.dma_start(out=feat_t[:, : t_s * EW], in_=src)
                    feat3 = feat_t[:, : t_s * EW].rearrange(
                        "p (i w) -> p i w", w=EW
                    )
                    oh_refs = build_onehots(feat3, t_s)
                    ps = psump.tile([BLK_V, 2 * F], f32)
                    for i in range(t_s):
                        oh, col0 = oh_refs[i]
                        nc.tensor.matmul(
                            ps[:],
                            lhsT=oh[:, col0 : col0 + BLK_V],
                            rhs=feat_t[:, i * EW : i * EW + 2 * F],
                            start=(i == 0),
                            stop=(i == t_s - 1),
                        )
                    # sum hi+lo halves: [128, (2,64)] -> [128, 64]
                    red = redp.tile([BLK_V, F], f32)
                    nc.vector.tensor_reduce(
                        red[:],
                        ps[:].rearrange("p (h f) -> p f h", h=2),
                        axis=mybir.AxisListType.X,
                        op=mybir.AluOpType.add,
                    )
                    # mean = sum * (1/deg) on ScalarE
                    nc.scalar.mul(
                        out_t[:, t * F : (t + 1) * F],
                        red[:],
                        rec[:, bo * T + t : bo * T + t + 1],
                    )
                nc.sync.dma_start(out=out_d[s], in_=out_t[:])

    nc.compile()
    return nc


def shard_table(indices, cfg=_DEFAULT_CFG):
    """Sort edges by destination and partition by core.

    Returns per-core list of (orig_edge_idx sorted by dest, block, rank_in_block,
    tiles_per_block)."""
    v = np.ascontiguousarray(indices[:, 1])
    order = np.argsort(v, kind="stable")
    vs = v[order]
    bounds = np.searchsorted(vs, np.arange(cfg.NCORES + 1) * cfg.VLOC)
    per_core = []
    for c in range(cfg.NCORES):
        lo, hi = bounds[c], bounds[c + 1]
        idx = order[lo:hi]
        vloc = vs[lo:hi].astype(np.int64) - c * cfg.VLOC
        blk = vloc // BLK_V
        vin = vloc % BLK_V
        cnt = np.bincount(blk, minlength=cfg.NBLK).astype(np.int64)
        starts = np.zeros(cfg.NBLK, dtype=np.int64)
        np.cumsum(cnt[:-1], out=starts[1:])
        rank = np.arange(len(idx), dtype=np.int64) - starts[blk]
        tiles = (cnt + BLK - 1) // BLK
        per_core.append((idx, blk, vin, rank, tiles))
    return per_core


def make_profile(per_core_tables, cfg=_DEFAULT_CFG):
    """Slot tile profile + per (core, table) block->slot permutation."""
    perms = []  # perms[t][c] = array: slot -> block
    sorted_tiles = []
    for per_core in per_core_tables:
        perms_t = []
        for c in range(cfg.NCORES):
            tiles = per_core[c][4]
            order = np.argsort(-tiles, kind="stable")
            perms_t.append(order)
            sorted_tiles.append(tiles[order])
        perms.append(perms_t)
    profile = np.max(np.stack(sorted_tiles), axis=0)
    profile = np.maximum(profile, 1)
    return [int(x) for x in profile], perms


def fill_feature_stream(per_core, features, profile, perm_t, cfg=_DEFAULT_CFG):
    """Per-core bf16 edge stream, slot-major, edge-slot-major within a slot.

    Row layout (130 bf16 words): [hi(64) | lo(64) | negv as f32 (2 words)].
    Padding rows have negv = +1.0 (never matches iota_neg <= 0)."""
    prof = np.asarray(profile, dtype=np.int64)
    row_base = np.concatenate([[0], np.cumsum(prof * BLK)]).astype(np.int64)
    total_rows = int(row_base[-1])

    hi = features.astype(ml_dtypes.bfloat16)
    lo = (features - hi.astype(np.float32)).astype(ml_dtypes.bfloat16)
    hi_u = hi.view(np.uint16)
    lo_u = lo.view(np.uint16)

    out = np.zeros((cfg.NCORES, total_rows, EW), dtype=np.uint16)
    out[:, :, 2 * F : 2 * F + 2] = ONE_F32_U16  # negv = +1.0 for padding rows
    for c in range(cfg.NCORES):
        idx, blk, vin, rank, _tiles = per_core[c]
        inv = np.empty(cfg.NBLK, dtype=np.int64)
        inv[perm_t[c]] = np.arange(cfg.NBLK)
        s = inv[blk]
        rows = row_base[s] + (rank & 127) * prof[s] + (rank >> 7)
        out[c, rows, 0:F] = hi_u[idx]
        out[c, rows, F : 2 * F] = lo_u[idx]
        out[c, rows, 2 * F : 2 * F + 2] = (
            (-vin.astype(np.float32)).view(np.uint32).view(np.uint16).reshape(-1, 2)
        )
    return out.reshape(cfg.NCORES, total_rows * EW).view(ml_dtypes.bfloat16)


def prep_adjacency(adjacency, perms, cfg=_DEFAULT_CFG):
    """adj_dev[c, g, vin, j*64 + t*32 + n] = adjacency[0, block_{t}(c, 7g+j), vin, t, n]
    padded with -1 beyond VLOC."""
    adj = np.ascontiguousarray(adjacency.reshape(cfg.V, T, N))
    adj_pad = np.full((cfg.NCORES, cfg.VPAD, T, N), -1, dtype=np.int32)
    adj_pad[:, : cfg.VLOC] = adj.reshape(cfg.NCORES, cfg.VLOC, T, N)
    adj_pad = adj_pad.reshape(cfg.NCORES, cfg.NBLK, BLK_V, T, N)
    out = np.empty((cfg.NCORES, cfg.NBLK, BLK_V, T, N), dtype=np.int32)
    for c in range(cfg.NCORES):
        for t in range(T):
            out[c, :, :, t, :] = adj_pad[c, perms[t][c], :, t, :]
    # [c, g, j, vin, t, n] -> [c, g, vin, j, t, n]
    out = out.reshape(cfg.NCORES, cfg.NBLK // ADJ_G, ADJ_G, BLK_V, T * N)
    out = np.ascontiguousarray(out.transpose(0, 1, 3, 2, 4))
    return out.reshape(cfg.NCORES, cfg.NBLK // ADJ_G, BLK_V, ADJ_G * T * N)


def prepare_inputs(adjacency, indices0, features0, indices1, features1, cfg=_DEFAULT_CFG):
    adjacency = np.asarray(adjacency)
    pc0 = shard_table(np.asarray(indices0), cfg)
    pc1 = shard_table(np.asarray(indices1), cfg)
    profile, perms = make_profile([pc0, pc1], cfg)

    f0 = fill_feature_stream(
        pc0, np.asarray(features0, dtype=np.float32), profile, perms[0], cfg
    )
    f1 = fill_feature_stream(
        pc1, np.asarray(features1, dtype=np.float32), profile, perms[1], cfg
    )
    adj = prep_adjacency(adjacency, perms, cfg)
    iota_neg = np.broadcast_to(
        -np.arange(BLK_V, dtype=np.float32), (BLK, BLK_V)
    ).copy()
    iota_pos = np.broadcast_to(
        np.arange(BLK_V).astype(ml_dtypes.bfloat16), (BLK, BLK_V)
    ).copy()

    in_maps = [
        {
            "feat0": f0[c],
            "feat1": f1[c],
            "adj": adj[c],
            "iota_neg": iota_neg,
            "iota_pos": iota_pos,
        }
        for c in range(cfg.NCORES)
    ]
    return in_maps, profile, perms


def assemble_output(core_outs, perms, cfg=_DEFAULT_CFG):
    outs = []
    for t in range(T):
        parts = []
        for c in range(cfg.NCORES):
            res_t = core_outs[c].reshape(cfg.NBLK, BLK_V, T, F)[:, :, t, :]
            tmp = np.empty((cfg.NBLK, BLK_V, F), dtype=res_t.dtype)
            tmp[perms[t][c]] = res_t
            parts.append(tmp.reshape(cfg.VPAD, F)[: cfg.VLOC])
        outs.append(np.concatenate(parts, axis=0).reshape(B, cfg.V, F))
    return (outs[0], outs[1])


def kernel(adjacency, indices0, features0, indices1, features1):
    from concourse.bass_utils import run_bass_kernel_spmd

    cfg = _DEFAULT_CFG
    in_maps, profile, perms = prepare_inputs(
        adjacency, indices0, features0, indices1, features1, cfg
    )

    key = tuple(profile)
    if key not in _NC_CACHE:
        _NC_CACHE[key] = build_device_program(profile, cfg)
    nc = _NC_CACHE[key]

    res = run_bass_kernel_spmd(nc, in_maps, list(range(cfg.NCORES)))
    return assemble_output(
        [res.results[c]["out"] for c in range(cfg.NCORES)], perms, cfg
    )



# revision 7
# speedup vs baseline: 1.4100x; 1.4100x over previous
"""Trainium2 Bass kernel for GNN mean aggregation (nn_AggrGSMean).

Computes, for t in {0,1}:
    out_t[b, v, :] = segment_sum(features_t over edges with dest v) / degree[b, v, t]
where degree[b, v, t] = max(count(adjacency[b, v, t, :] >= 0), 1).

Strategy (graph-partition sharding per the problem's sharding hint):
- Host: partition edges by destination-vertex range across 8 cores, sort each
  core's edges by destination.  Edges of the same destination are PAIRED
  (Q=2, odd counts padded with a zero edge); each pair-row carries the two
  edges' features interleaved feature-major (f0e0 f0e1 f1e0 ...) in bf16.
  Pair-rows are grouped into 128-vertex blocks; each block's pair list is
  padded to whole 128-row tiles.  Blocks are slot-assigned in decreasing
  tile-count order so one static per-slot profile (max over cores/tables)
  serves all cores.  The destination slot-vertex of each pair-row ships as a
  separate bf16 "vin" stream [128, total_tiles]; reciprocal degrees are
  computed on host and shipped as f32 [128, NBLK*T].
- Device (per core): per (slot, t): DMA the slot's pair tiles [128, t_s*128]
  bf16; one-hot [128 pair-rows x 128 vslots] built by iota==vin (batched k
  tiles per DVE instruction, an act_frac share on ScalarE via
  relu(1-(iota-v)^2)); per tile one matmul accumulates onehot.T @ raw pairs
  into PSUM [128, (f,q)=128] f32; DVE adds the two q columns per f (one
  reduce per slot), ScalarE multiplies by the resident 1/degree column and
  writes a bf16 group output tile, DMA'd out once per 7-slot group.
"""

import sys

if "/opt/trn_rl_repo" not in sys.path:
    sys.path.insert(0, "/opt/trn_rl_repo")

import ml_dtypes
import numpy as np

# Problem constants (hardcoded per contract)
B, V, T, N, F, M = 1, 100000, 2, 32, 64, 1600000
NCORES = 8
BLK = 128           # pair-rows per tile (matmul contraction)
BLK_V = 128         # vertices per block / one-hot width
Q = 2               # edges pre-summed per pair-row
ROW_W = Q * F       # bf16 words per pair-row (128)
G = 7               # slots per output group
VLOC = V // NCORES          # 12500
NBLK = -(-VLOC // BLK_V)    # 98
NGRP = -(-NBLK // G)        # 14
VPAD = NBLK * BLK_V         # 12544


class Cfg:
    def __init__(self):
        self.V = V
        self.NCORES = NCORES
        self.VLOC = VLOC
        self.NBLK = NBLK
        self.VPAD = VPAD


_DEFAULT_CFG = Cfg()
_NC_CACHE = {}


def build_device_program(profile, cfg=_DEFAULT_CFG, act_frac=0.16, oh_chunk=8):
    """Build + compile the per-core Bass program.

    profile: per-slot tile counts (len NBLK); same static schedule on all
    cores.  One-hot builds are batched (k tiles per instruction) on DVE with
    an act_frac fraction routed to ScalarE via relu(1-(iota-v)^2)."""
    from contextlib import ExitStack

    import concourse.tile as tile
    from concourse import bacc, mybir

    f32 = mybir.dt.float32
    bf16 = mybir.dt.bfloat16
    assert len(profile) == NBLK
    prof = np.asarray(profile, dtype=np.int64)
    t_max = int(prof.max())
    TT = int(prof.sum())                      # total tiles per table
    sb = np.concatenate([[0], np.cumsum(prof * BLK * ROW_W)]).astype(np.int64)
    tb = np.concatenate([[0], np.cumsum(prof)]).astype(np.int64)
    total_elems = int(sb[-1])

    nc = bacc.Bacc("TRN2", target_bir_lowering=False, debug=False)
    feat_d = [
        nc.dram_tensor(f"feat{t}", [total_elems], bf16, kind="ExternalInput").ap()
        for t in range(T)
    ]
    vin_d = [
        nc.dram_tensor(f"vin{t}", [BLK, TT], bf16, kind="ExternalInput").ap()
        for t in range(T)
    ]
    rec_d = nc.dram_tensor("rec", [BLK_V, NBLK * T], f32, kind="ExternalInput").ap()
    iota_d = nc.dram_tensor("iota", [BLK, BLK_V], bf16, kind="ExternalInput").ap()
    out_d = nc.dram_tensor(
        "out", [NGRP, BLK_V, G * T * F], bf16, kind="ExternalOutput"
    ).ap()

    with tile.TileContext(nc) as tc, ExitStack() as ctx:
        const = ctx.enter_context(tc.tile_pool(name="const", bufs=1))
        featp = ctx.enter_context(tc.tile_pool(name="featp", bufs=6))
        ohdp = ctx.enter_context(tc.tile_pool(name="ohdp", bufs=5))
        ohap = ctx.enter_context(tc.tile_pool(name="ohap", bufs=7))
        redp = ctx.enter_context(tc.tile_pool(name="redp", bufs=5))
        outp = ctx.enter_context(tc.tile_pool(name="outp", bufs=3))
        psump = ctx.enter_context(tc.tile_pool(name="psum", bufs=8, space="PSUM"))

        # Const loads ride the Activation engine's HWDGE queues so they can
        # never get stuck behind feature DMAs (sync queues) that wait on
        # tile-pool recycling.
        iota_t = const.tile([BLK, BLK_V], bf16)
        nc.scalar.dma_start(out=iota_t[:], in_=iota_d[:])
        rec_t = const.tile([BLK_V, NBLK * T], f32)
        nc.scalar.dma_start(out=rec_t[:], in_=rec_d[:])
        vin_t = []
        vinf_t = []
        for t in range(T):
            vt = const.tile([BLK, TT], bf16, tag=f"vin{t}")
            # 4-chunk column split so the load spreads across DMA queues
            bnds = [TT * i // 4 for i in range(5)]
            for a, b in zip(bnds[:-1], bnds[1:]):
                if b > a:
                    nc.scalar.dma_start(out=vt[:, a:b], in_=vin_d[t][:, a:b])
            vin_t.append(vt)
            # f32 negated copy for the ScalarE activation-bias one-hot path
            vf = const.tile([BLK, TT], f32, tag=f"vinf{t}")
            nc.vector.tensor_scalar(
                vf[:], vt[:], -1.0, None, op0=mybir.AluOpType.mult
            )
            vinf_t.append(vf)

        # --- one-hot build planning: weighted chunk assignment across engines
        n_act = [0]
        n_tot = [0]

        def plan_chunks(t_s):
            chunks = []
            i = 0
            while i < t_s:
                if n_act[0] < act_frac * n_tot[0]:
                    k = 1
                    chunks.append(("act", i, k))
                    n_act[0] += 1
                else:
                    k = min(oh_chunk, t_s - i)
                    chunks.append(("dve", i, k))
                i += k
                n_tot[0] += k
            return chunks

        for g in range(NGRP):
            out_t = outp.tile([BLK_V, G * T * F], bf16)
            for so in range(G):
                s = g * G + so
                t_s = int(prof[s])
                for t in range(T):
                    feat_t = featp.tile([BLK, t_max * ROW_W], bf16, tag="feat")
                    src = feat_d[t][int(sb[s]) : int(sb[s + 1])].rearrange(
                        "(e w) -> e w", w=t_s * ROW_W
                    )
                    nc.sync.dma_start(out=feat_t[:, : t_s * ROW_W], in_=src)
                    # one-hot tiles: batched DVE is_equal + ScalarE act share
                    refs = [None] * t_s
                    for kind, i0, k in plan_chunks(t_s):
                        if kind == "act":
                            bias = vinf_t[t][:, int(tb[s]) + i0 : int(tb[s]) + i0 + 1]
                            y = ohap.tile([BLK, BLK_V], bf16, tag="y")
                            nc.scalar.activation(
                                y[:], iota_t[:],
                                mybir.ActivationFunctionType.Square,
                                bias=bias, scale=1.0,
                            )
                            oh = ohap.tile([BLK, BLK_V], bf16, tag="oha")
                            nc.scalar.activation(
                                oh[:], y[:], mybir.ActivationFunctionType.Relu,
                                bias=1.0, scale=-1.0,
                            )
                            refs[i0] = (oh, 0)
                        else:
                            oh = ohdp.tile([BLK, oh_chunk * BLK_V], bf16, tag="ohd")
                            oh3 = oh[:, : k * BLK_V].rearrange(
                                "p (i v) -> p i v", v=BLK_V
                            )
                            in0 = iota_t[:].unsqueeze(1).broadcast_to([BLK, k, BLK_V])
                            in1 = (
                                vin_t[t][:, int(tb[s]) + i0 : int(tb[s]) + i0 + k]
                                .unsqueeze(2)
                                .broadcast_to([BLK, k, BLK_V])
                            )
                            nc.vector.tensor_tensor(
                                oh3, in0, in1, op=mybir.AluOpType.is_equal
                            )
                            for j in range(k):
                                refs[i0 + j] = (oh, j * BLK_V)
                    ps = psump.tile([BLK_V, ROW_W], f32)
                    for i in range(t_s):
                        oh, col0 = refs[i]
                        nc.tensor.matmul(
                            ps[:],
                            lhsT=oh[:, col0 : col0 + BLK_V],
                            rhs=feat_t[:, i * ROW_W : (i + 1) * ROW_W],
                            start=(i == 0),
                            stop=(i == t_s - 1),
                        )
                    # pair post-add: psum [v, (f q)] -> red [v, f] in f32
                    red = redp.tile([BLK_V, F], f32)
                    nc.vector.tensor_reduce(
                        red[:],
                        ps[:].rearrange("p (f q) -> p f q", q=Q),
                        axis=mybir.AxisListType.X,
                        op=mybir.AluOpType.add,
                    )
                    nc.scalar.mul(
                        out_t[:, (so * T + t) * F : (so * T + t + 1) * F],
                        red[:],
                        rec_t[:, s * T + t : s * T + t + 1],
                    )
            nc.scalar.dma_start(out=out_d[g], in_=out_t[:])

    nc.compile()
    return nc


def shard_table(indices, cfg=_DEFAULT_CFG):
    """Per-edge placement: sort by destination, pair same-dest edges.

    Returns (order, core, slot-free fields) needed by make_profile/fill:
      dict with per-edge (sorted order) arrays and per-(core,block) tile counts.
    """
    v = np.ascontiguousarray(indices[:, 1]).astype(np.int64)
    order = np.argsort(v, kind="stable")
    vs = v[order]
    # per-vertex counts and ranks
    n_v = np.bincount(vs, minlength=V)
    starts = np.concatenate([[0], np.cumsum(n_v)])
    r = np.arange(len(vs), dtype=np.int64) - starts[vs]
    pv = (n_v + 1) // 2                                  # pairs per vertex
    pb = np.concatenate([[0], np.cumsum(pv)])            # pair id base per vertex
    core = vs // VLOC
    vloc = vs % VLOC
    blk = vloc // BLK_V
    vin = vloc % BLK_V
    # pair-row rank within (core, block)
    bps = pb[(np.arange(NCORES)[:, None] * VLOC + np.arange(NBLK)[None, :] * BLK_V)]
    pr = (pb[vs] - bps[core, blk]) + r // 2
    # per (core, block) pair counts -> tiles
    pv_pad = np.zeros(NCORES * VPAD, dtype=np.int64)
    pv_pad_idx = np.arange(V)
    pv_pad[(pv_pad_idx // VLOC) * VPAD + (pv_pad_idx % VLOC)] = pv
    cb = pv_pad.reshape(NCORES, NBLK, BLK_V).sum(axis=2)
    tiles = -(-cb // BLK)                                # [NCORES, NBLK]
    return {
        "order": order, "core": core, "blk": blk, "vin": vin,
        "pr": pr, "q": (r & 1).astype(np.int64), "tiles": tiles,
    }


def make_profile(tables):
    """Shared slot tile profile + per (table, core) slot->block permutation."""
    perms = []
    sorted_tiles = []
    for tab in tables:
        perms_t = []
        for c in range(NCORES):
            tl = tab["tiles"][c]
            p = np.argsort(-tl, kind="stable")
            perms_t.append(p)
            sorted_tiles.append(tl[p])
        perms.append(np.stack(perms_t))
    profile = np.max(np.stack(sorted_tiles), axis=0)
    profile = np.maximum(profile, 1)
    return [int(x) for x in profile], perms


def fill_streams(tab, features, profile, perm, cfg=_DEFAULT_CFG):
    """Build per-core bf16 feature stream + vin stream for one table."""
    prof = np.asarray(profile, dtype=np.int64)
    TT = int(prof.sum())
    sb = np.concatenate([[0], np.cumsum(prof * BLK * ROW_W)]).astype(np.int64)
    tb = np.concatenate([[0], np.cumsum(prof)]).astype(np.int64)
    TW = int(sb[-1])

    inv = np.empty((NCORES, NBLK), dtype=np.int64)
    for c in range(NCORES):
        inv[c, perm[c]] = np.arange(NBLK)

    hi = features.astype(ml_dtypes.bfloat16)
    hi_u = hi.view(np.uint16)

    core = tab["core"]
    s_e = inv[core, tab["blk"]]
    p = tab["pr"] & (BLK - 1)
    i = tab["pr"] >> 7
    q = tab["q"]
    pos = core * TW + sb[s_e] + p * (prof[s_e] * ROW_W) + i * ROW_W + q * 1
    # feature word layout within row: word f*Q + q
    stream = np.zeros(NCORES * TW, dtype=np.uint16)
    cols = (Q * np.arange(F, dtype=np.int64))[None, :]
    stream[pos[:, None] + cols] = hi_u[tab["order"]]
    stream = stream.reshape(NCORES, TW).view(ml_dtypes.bfloat16)

    # vin stream [NCORES, 128, TT]; padding rows get -1 (never matches iota)
    vin_arr = np.full(NCORES * BLK * TT, -1.0, dtype=ml_dtypes.bfloat16)
    m0 = q == 0
    flat = core[m0] * (BLK * TT) + p[m0] * TT + (tb[s_e[m0]] + i[m0])
    vin_arr[flat] = tab["vin"][m0].astype(ml_dtypes.bfloat16)
    vin_arr = vin_arr.reshape(NCORES, BLK, TT)
    return stream, vin_arr


def prep_rec(adjacency, perms, cfg=_DEFAULT_CFG):
    """rec[c][vin, s*T + t] = 1/degree(core c, block perm[t][c][s], vin, t)."""
    adj = np.asarray(adjacency).reshape(V, T, N)
    deg = np.maximum((adj >= 0).sum(axis=-1), 1).astype(np.float64)  # [V, T]
    rec_full = (1.0 / deg).astype(np.float32)
    rec_pad = np.ones((NCORES, VPAD, T), dtype=np.float32)
    rec_pad[:, :VLOC] = rec_full.reshape(NCORES, VLOC, T)
    rec_pad = rec_pad.reshape(NCORES, NBLK, BLK_V, T)
    out = np.empty((NCORES, BLK_V, NBLK, T), dtype=np.float32)
    for c in range(NCORES):
        for t in range(T):
            out[c, :, :, t] = rec_pad[c, perms[t][c], :, t].T
    return out.reshape(NCORES, BLK_V, NBLK * T)


def prepare_inputs(adjacency, indices0, features0, indices1, features1, cfg=_DEFAULT_CFG):
    tab0 = shard_table(np.asarray(indices0), cfg)
    tab1 = shard_table(np.asarray(indices1), cfg)
    profile, perms = make_profile([tab0, tab1])

    f0, v0 = fill_streams(tab0, np.asarray(features0, dtype=np.float32), profile, perms[0], cfg)
    f1, v1 = fill_streams(tab1, np.asarray(features1, dtype=np.float32), profile, perms[1], cfg)
    rec = prep_rec(adjacency, perms, cfg)
    iota = np.broadcast_to(
        np.arange(BLK_V).astype(ml_dtypes.bfloat16), (BLK, BLK_V)
    ).copy()

    in_maps = [
        {
            "feat0": f0[c],
            "feat1": f1[c],
            "vin0": v0[c],
            "vin1": v1[c],
            "rec": rec[c],
            "iota": iota,
        }
        for c in range(NCORES)
    ]
    return in_maps, profile, perms


def assemble_output(core_outs, perms, cfg=_DEFAULT_CFG):
    outs = []
    for t in range(T):
        parts = []
        for c in range(NCORES):
            arr = np.asarray(core_outs[c]).astype(np.float32)
            arr = arr.reshape(NGRP, BLK_V, G, T, F)[:, :, :, t, :]
            arr = arr.transpose(0, 2, 1, 3).reshape(NGRP * G, BLK_V, F)[:NBLK]
            tmp = np.empty((NBLK, BLK_V, F), dtype=np.float32)
            tmp[perms[t][c]] = arr
            parts.append(tmp.reshape(VPAD, F)[:VLOC])
        outs.append(np.concatenate(parts, axis=0).reshape(B, V, F))
    return (outs[0], outs[1])


def kernel(adjacency, indices0, features0, indices1, features1):
    from concourse.bass_utils import run_bass_kernel_spmd

    cfg = _DEFAULT_CFG
    in_maps, profile, perms = prepare_inputs(
        adjacency, indices0, features0, indices1, features1, cfg
    )

    key = tuple(profile)
    if key not in _NC_CACHE:
        _NC_CACHE[key] = build_device_program(profile, cfg)
    nc = _NC_CACHE[key]

    res = run_bass_kernel_spmd(nc, in_maps, list(range(NCORES)))
    return assemble_output(
        [res.results[c]["out"] for c in range(NCORES)], perms, cfg
    )


# revision 10
# speedup vs baseline: 2.1495x; 1.5245x over previous
"""Trainium2 Bass kernel for GNN mean aggregation (nn_AggrGSMean).

Computes, for t in {0,1}:
    out_t[b, v, :] = segment_sum(features_t over edges with dest v) / degree[b, v, t]
where degree[b, v, t] = max(count(adjacency[b, v, t, :] >= 0), 1).

Strategy (graph-partition sharding per the problem's sharding hint):
- Host: partition edges by destination-vertex range across 8 cores, sort each
  core's edges by destination.  Edges of the same destination are PAIRED
  (Q=2, odd counts padded with a zero edge); each pair-row carries the two
  edges' features interleaved feature-major (f0e0 f0e1 f1e0 ...) in bf16.
  Pair-rows are grouped into 128-vertex blocks; each block's pair list is
  padded to whole 128-row tiles.  Blocks are slot-assigned in decreasing
  tile-count order so one static per-slot profile (max over cores/tables)
  serves all cores.  The destination slot-vertex of each pair-row ships as a
  separate bf16 "vin" stream [128, total_tiles]; reciprocal degrees are
  computed on host and shipped as f32 [128, NBLK*T].
- Device (per core): per (slot, t): DMA the slot's pair tiles [128, t_s*128]
  bf16; one-hot [128 pair-rows x 128 vslots] built by iota==vin (batched k
  tiles per DVE instruction, an act_frac share on ScalarE via
  relu(1-(iota-v)^2)); per tile one matmul accumulates onehot.T @ raw pairs
  into PSUM [128, (f,q)=128] f32; DVE adds the two q columns per f (one
  reduce per slot), ScalarE multiplies by the resident 1/degree column and
  writes a bf16 group output tile, DMA'd out once per 7-slot group.
"""

import sys

if "/opt/trn_rl_repo" not in sys.path:
    sys.path.insert(0, "/opt/trn_rl_repo")

import ml_dtypes
import numpy as np

# Problem constants (hardcoded per contract)
B, V, T, N, F, M = 1, 100000, 2, 32, 64, 1600000
NCORES = 8
BLK = 128           # pair-rows per tile (matmul contraction)
BLK_V = 128         # vertices per block / one-hot width
Q = 2               # edges pre-summed per pair-row
ROW_W = Q * F       # bf16 words per pair-row (128)
G = 7               # slots per output group
VLOC = V // NCORES          # 12500
NBLK = -(-VLOC // BLK_V)    # 98
NGRP = -(-NBLK // G)        # 14
VPAD = NBLK * BLK_V         # 12544


class Cfg:
    def __init__(self):
        self.V = V
        self.NCORES = NCORES
        self.VLOC = VLOC
        self.NBLK = NBLK
        self.VPAD = VPAD


_DEFAULT_CFG = Cfg()
_NC_CACHE = {}


def build_device_program(
    profile, cfg=_DEFAULT_CFG, act_frac=0.10, b_frac=0.55, oh_chunk=8, ring_mod=4
):
    """Build + compile the per-core Bass program.

    One-hot builds are batched k tiles per DVE instruction in two layouts:
    'a' = (i v) broadcast-last (slower DVE, contiguous LDWEIGHTS) and
    'b' = (v i) all-packed transposed (faster DVE, strided LDWEIGHTS);
    b_frac balances DVE vs PE load.  act_frac of tiles go to ScalarE via
    relu(1-(iota-v)^2).  Every ring_mod'th feature DMA rides the Activation
    HWDGE ring for extra DMA bandwidth."""
    from contextlib import ExitStack

    import concourse.tile as tile
    from concourse import bacc, mybir

    f32 = mybir.dt.float32
    bf16 = mybir.dt.bfloat16
    assert len(profile) == NBLK
    prof = np.asarray(profile, dtype=np.int64)
    t_max = int(prof.max())
    TT = int(prof.sum())                      # total tiles per table
    sb = np.concatenate([[0], np.cumsum(prof * BLK * ROW_W)]).astype(np.int64)
    tb = np.concatenate([[0], np.cumsum(prof)]).astype(np.int64)
    total_elems = int(sb[-1])

    nc = bacc.Bacc("TRN2", target_bir_lowering=False, debug=False)
    feat_d = [
        nc.dram_tensor(f"feat{t}", [total_elems], bf16, kind="ExternalInput").ap()
        for t in range(T)
    ]
    vin_d = [
        nc.dram_tensor(f"vin{t}", [BLK, TT], bf16, kind="ExternalInput").ap()
        for t in range(T)
    ]
    iota_d = nc.dram_tensor("iota", [BLK, BLK_V], bf16, kind="ExternalInput").ap()
    iotar_d = nc.dram_tensor(
        "iotar", [BLK, oh_chunk * BLK_V], bf16, kind="ExternalInput"
    ).ap()
    out_d = nc.dram_tensor(
        "out", [NGRP, BLK_V, G * T * F], bf16, kind="ExternalOutput"
    ).ap()

    with tile.TileContext(nc) as tc, ExitStack() as ctx:
        const = ctx.enter_context(tc.tile_pool(name="const", bufs=1))
        featp = ctx.enter_context(tc.tile_pool(name="featp", bufs=8))
        ohdp = ctx.enter_context(tc.tile_pool(name="ohdp", bufs=6))
        ohap = ctx.enter_context(tc.tile_pool(name="ohap", bufs=7))
        outp = ctx.enter_context(tc.tile_pool(name="outp", bufs=3))
        psump = ctx.enter_context(tc.tile_pool(name="psum", bufs=8, space="PSUM"))

        # Const loads ride the Activation engine's HWDGE queues so they can
        # never get stuck behind feature DMAs (sync queues) that wait on
        # tile-pool recycling.
        iota_t = const.tile([BLK, BLK_V], bf16)
        nc.scalar.dma_start(out=iota_t[:], in_=iota_d[:])
        iotar_t = const.tile([BLK, oh_chunk * BLK_V], bf16)
        nc.scalar.dma_start(out=iotar_t[:], in_=iotar_d[:])
        vin_t = []
        vinf_t = []
        for t in range(T):
            vt = const.tile([BLK, TT], bf16, tag=f"vin{t}")
            # 4-chunk column split so the load spreads across DMA queues
            bnds = [TT * i // 4 for i in range(5)]
            for a, b in zip(bnds[:-1], bnds[1:]):
                if b > a:
                    nc.scalar.dma_start(out=vt[:, a:b], in_=vin_d[t][:, a:b])
            vin_t.append(vt)
            # f32 negated copy for the ScalarE activation-bias one-hot path
            vf = const.tile([BLK, TT], f32, tag=f"vinf{t}")
            nc.vector.tensor_scalar(
                vf[:], vt[:], -1.0, None, op0=mybir.AluOpType.mult
            )
            vinf_t.append(vf)

        # --- one-hot build planning: weighted chunk assignment
        n_act = [0]
        n_b = [0]
        n_tot = [0]

        def plan_chunks(t_s):
            chunks = []
            i = 0
            while i < t_s:
                rest = t_s - i
                if n_act[0] < act_frac * n_tot[0]:
                    k = 1
                    chunks.append(("act", i, k))
                    n_act[0] += 1
                elif rest >= oh_chunk and n_b[0] < b_frac * n_tot[0]:
                    k = oh_chunk
                    chunks.append(("b", i, k))
                    n_b[0] += k
                else:
                    k = min(oh_chunk, rest)
                    chunks.append(("a", i, k))
                i += k
                n_tot[0] += k
            return chunks

        ndma = [0]
        for g in range(NGRP):
            out_t = outp.tile([BLK_V, G * T * F], bf16)
            for so in range(G):
                s = g * G + so
                t_s = int(prof[s])
                for t in range(T):
                    feat_t = featp.tile([BLK, t_max * ROW_W], bf16, tag="feat")
                    src = feat_d[t][int(sb[s]) : int(sb[s + 1])].rearrange(
                        "(e w) -> e w", w=t_s * ROW_W
                    )
                    ndma[0] += 1
                    deng = nc.scalar if ndma[0] % ring_mod == 0 else nc.sync
                    deng.dma_start(out=feat_t[:, : t_s * ROW_W], in_=src)
                    # one-hot tiles: per-chunk engine/layout assignment
                    refs = [None] * t_s
                    for kind, i0, k in plan_chunks(t_s):
                        c0 = int(tb[s]) + i0
                        if kind == "act":
                            bias = vinf_t[t][:, c0 : c0 + 1]
                            y = ohap.tile([BLK, BLK_V], bf16, tag="y")
                            nc.scalar.activation(
                                y[:], iota_t[:],
                                mybir.ActivationFunctionType.Square,
                                bias=bias, scale=1.0,
                            )
                            oh = ohap.tile([BLK, BLK_V], bf16, tag="oha")
                            nc.scalar.activation(
                                oh[:], y[:], mybir.ActivationFunctionType.Relu,
                                bias=1.0, scale=-1.0,
                            )
                            refs[i0] = (oh, 0, 1)
                        elif kind == "b":
                            oh = ohdp.tile([BLK, oh_chunk * BLK_V], bf16, tag="ohb")
                            oh3 = oh[:, : k * BLK_V].rearrange(
                                "p (v i) -> p v i", i=k
                            )
                            in0 = iotar_t[:].rearrange(
                                "p (v i) -> p v i", i=oh_chunk
                            )[:, :, :k] if k != oh_chunk else iotar_t[:].rearrange(
                                "p (v i) -> p v i", i=k
                            )
                            in1 = (
                                vin_t[t][:, c0 : c0 + k]
                                .unsqueeze(1)
                                .broadcast_to([BLK, BLK_V, k])
                            )
                            nc.vector.tensor_tensor(
                                oh3, in0, in1, op=mybir.AluOpType.is_equal
                            )
                            for j in range(k):
                                refs[i0 + j] = (oh, j, k)
                        else:
                            oh = ohdp.tile([BLK, oh_chunk * BLK_V], bf16, tag="oha2")
                            oh3 = oh[:, : k * BLK_V].rearrange(
                                "p (i v) -> p i v", v=BLK_V
                            )
                            in0 = iota_t[:].unsqueeze(1).broadcast_to([BLK, k, BLK_V])
                            in1 = (
                                vin_t[t][:, c0 : c0 + k]
                                .unsqueeze(2)
                                .broadcast_to([BLK, k, BLK_V])
                            )
                            nc.vector.tensor_tensor(
                                oh3, in0, in1, op=mybir.AluOpType.is_equal
                            )
                            for j in range(k):
                                refs[i0 + j] = (oh, j * BLK_V, 1)
                    ps = psump.tile([BLK_V, ROW_W], f32)
                    for i in range(t_s):
                        oh, idx, stride = refs[i]
                        if stride == 1:
                            lhsT = oh[:, idx : idx + BLK_V]
                        else:
                            lhsT = oh[:, : stride * BLK_V].rearrange(
                                "p (v i) -> p i v", i=stride
                            )[:, idx, :]
                        nc.tensor.matmul(
                            ps[:],
                            lhsT=lhsT,
                            rhs=feat_t[:, i * ROW_W : (i + 1) * ROW_W],
                            start=(i == 0),
                            stop=(i == t_s - 1),
                        )
                    # pair post-add: reduce psum (q f) over q -> bf16 out column
                    with nc.allow_low_precision(reason="bf16 mean output"):
                        nc.vector.tensor_reduce(
                            out_t[:, (so * T + t) * F : (so * T + t + 1) * F],
                            ps[:].rearrange("p (q f) -> p f q", q=Q),
                            axis=mybir.AxisListType.X,
                            op=mybir.AluOpType.add,
                        )
            nc.scalar.dma_start(out=out_d[g], in_=out_t[:])

    nc.compile()
    return nc


def shard_table(indices, cfg=_DEFAULT_CFG):
    """Per-edge placement: sort by destination, pair same-dest edges.

    Returns (order, core, slot-free fields) needed by make_profile/fill:
      dict with per-edge (sorted order) arrays and per-(core,block) tile counts.
    """
    v = np.ascontiguousarray(indices[:, 1]).astype(np.int64)
    order = np.argsort(v, kind="stable")
    vs = v[order]
    # per-vertex counts and ranks
    n_v = np.bincount(vs, minlength=V)
    starts = np.concatenate([[0], np.cumsum(n_v)])
    r = np.arange(len(vs), dtype=np.int64) - starts[vs]
    pv = (n_v + 1) // 2                                  # pairs per vertex
    pb = np.concatenate([[0], np.cumsum(pv)])            # pair id base per vertex
    core = vs // VLOC
    vloc = vs % VLOC
    blk = vloc // BLK_V
    vin = vloc % BLK_V
    # pair-row rank within (core, block)
    bps = pb[(np.arange(NCORES)[:, None] * VLOC + np.arange(NBLK)[None, :] * BLK_V)]
    pr = (pb[vs] - bps[core, blk]) + r // 2
    # per (core, block) pair counts -> tiles
    pv_pad = np.zeros(NCORES * VPAD, dtype=np.int64)
    pv_pad_idx = np.arange(V)
    pv_pad[(pv_pad_idx // VLOC) * VPAD + (pv_pad_idx % VLOC)] = pv
    cb = pv_pad.reshape(NCORES, NBLK, BLK_V).sum(axis=2)
    tiles = -(-cb // BLK)                                # [NCORES, NBLK]
    return {
        "order": order, "core": core, "blk": blk, "vin": vin,
        "pr": pr, "q": (r & 1).astype(np.int64), "tiles": tiles,
    }


def make_profile(tables):
    """Shared slot tile profile + per (table, core) slot->block permutation."""
    perms = []
    sorted_tiles = []
    for tab in tables:
        perms_t = []
        for c in range(NCORES):
            tl = tab["tiles"][c]
            p = np.argsort(-tl, kind="stable")
            perms_t.append(p)
            sorted_tiles.append(tl[p])
        perms.append(np.stack(perms_t))
    profile = np.max(np.stack(sorted_tiles), axis=0)
    profile = np.maximum(profile, 1)
    return [int(x) for x in profile], perms


def fill_streams(tab, features, rec_e, profile, perm, cfg=_DEFAULT_CFG):
    """Per-core bf16 feature stream (pre-scaled by 1/degree) + vin stream.

    Row layout is (q f): word q*F + f, so the PSUM pair halves are the
    contiguous column blocks [0:F] and [F:2F]."""
    prof = np.asarray(profile, dtype=np.int64)
    TT = int(prof.sum())
    sb = np.concatenate([[0], np.cumsum(prof * BLK * ROW_W)]).astype(np.int64)
    tb = np.concatenate([[0], np.cumsum(prof)]).astype(np.int64)
    TW = int(sb[-1])

    inv = np.empty((NCORES, NBLK), dtype=np.int64)
    for c in range(NCORES):
        inv[c, perm[c]] = np.arange(NBLK)

    scaled = features[tab["order"]] * rec_e[:, None]
    hi_u = scaled.astype(ml_dtypes.bfloat16).view(np.uint16)

    core = tab["core"]
    s_e = inv[core, tab["blk"]]
    p = tab["pr"] & (BLK - 1)
    i = tab["pr"] >> 7
    q = tab["q"]
    pos = core * TW + sb[s_e] + p * (prof[s_e] * ROW_W) + i * ROW_W + q * F
    stream = np.zeros(NCORES * TW, dtype=np.uint16)
    cols = np.arange(F, dtype=np.int64)[None, :]
    stream[pos[:, None] + cols] = hi_u
    stream = stream.reshape(NCORES, TW).view(ml_dtypes.bfloat16)

    # vin stream [NCORES, 128, TT]; padding rows get -1 (never matches iota)
    vin_arr = np.full(NCORES * BLK * TT, -1.0, dtype=ml_dtypes.bfloat16)
    m0 = q == 0
    flat = core[m0] * (BLK * TT) + p[m0] * TT + (tb[s_e[m0]] + i[m0])
    vin_arr[flat] = tab["vin"][m0].astype(ml_dtypes.bfloat16)
    vin_arr = vin_arr.reshape(NCORES, BLK, TT)
    return stream, vin_arr


def edge_recip(adjacency, tab, t):
    """1/degree at each sorted edge's destination for table t."""
    adj = np.asarray(adjacency).reshape(V, T, N)
    deg = np.maximum((adj[:, t] >= 0).sum(axis=-1), 1).astype(np.float64)  # [V]
    rec = (1.0 / deg).astype(np.float32)
    vs = (tab["core"] * VLOC + tab["blk"] * BLK_V + tab["vin"]).astype(np.int64)
    return rec[vs]


def prepare_inputs(adjacency, indices0, features0, indices1, features1, cfg=_DEFAULT_CFG, oh_chunk=8):
    tab0 = shard_table(np.asarray(indices0), cfg)
    tab1 = shard_table(np.asarray(indices1), cfg)
    profile, perms = make_profile([tab0, tab1])

    r0 = edge_recip(adjacency, tab0, 0)
    r1 = edge_recip(adjacency, tab1, 1)
    f0, v0 = fill_streams(tab0, np.asarray(features0, dtype=np.float32), r0, profile, perms[0], cfg)
    f1, v1 = fill_streams(tab1, np.asarray(features1, dtype=np.float32), r1, profile, perms[1], cfg)
    iota = np.broadcast_to(
        np.arange(BLK_V).astype(ml_dtypes.bfloat16), (BLK, BLK_V)
    ).copy()
    iotar = np.broadcast_to(
        (np.arange(oh_chunk * BLK_V) // oh_chunk).astype(ml_dtypes.bfloat16),
        (BLK, oh_chunk * BLK_V),
    ).copy()

    in_maps = [
        {
            "feat0": f0[c],
            "feat1": f1[c],
            "vin0": v0[c],
            "vin1": v1[c],
            "iota": iota,
            "iotar": iotar,
        }
        for c in range(NCORES)
    ]
    return in_maps, profile, perms


def assemble_output(core_outs, perms, cfg=_DEFAULT_CFG):
    outs = []
    for t in range(T):
        parts = []
        for c in range(NCORES):
            arr = np.asarray(core_outs[c]).astype(np.float32)
            arr = arr.reshape(NGRP, BLK_V, G, T, F)[:, :, :, t, :]
            arr = arr.transpose(0, 2, 1, 3).reshape(NGRP * G, BLK_V, F)[:NBLK]
            tmp = np.empty((NBLK, BLK_V, F), dtype=np.float32)
            tmp[perms[t][c]] = arr
            parts.append(tmp.reshape(VPAD, F)[:VLOC])
        outs.append(np.concatenate(parts, axis=0).reshape(B, V, F))
    return (outs[0], outs[1])


def kernel(adjacency, indices0, features0, indices1, features1):
    from concourse.bass_utils import run_bass_kernel_spmd

    cfg = _DEFAULT_CFG
    in_maps, profile, perms = prepare_inputs(
        adjacency, indices0, features0, indices1, features1, cfg
    )

    key = tuple(profile)
    if key not in _NC_CACHE:
        _NC_CACHE[key] = build_device_program(profile, cfg)
    nc = _NC_CACHE[key]

    res = run_bass_kernel_spmd(nc, in_maps, list(range(NCORES)))
    return assemble_output(
        [res.results[c]["out"] for c in range(NCORES)], perms, cfg
    )


# revision 15
# speedup vs baseline: 2.3113x; 1.0753x over previous
"""Trainium2 Bass kernel for GNN mean aggregation (nn_AggrGSMean).

Computes, for t in {0,1}:
    out_t[b, v, :] = segment_sum(features_t over edges with dest v) / degree[b, v, t]
where degree[b, v, t] = max(count(adjacency[b, v, t, :] >= 0), 1).

Strategy (graph-partition sharding per the problem's sharding hint):
- Host: partition edges by destination-vertex range across 8 cores, sort each
  core's edges by destination.  Edges of the same destination are PAIRED
  (Q=2, odd counts padded with a zero edge); each pair-row carries the two
  edges' features interleaved feature-major (f0e0 f0e1 f1e0 ...) in bf16.
  Pair-rows are grouped into 128-vertex blocks; each block's pair list is
  padded to whole 128-row tiles.  Blocks are slot-assigned in decreasing
  tile-count order so one static per-slot profile (max over cores/tables)
  serves all cores.  The destination slot-vertex of each pair-row ships as a
  separate bf16 "vin" stream [128, total_tiles]; reciprocal degrees are
  computed on host and shipped as f32 [128, NBLK*T].
- Device (per core): per (slot, t): DMA the slot's pair tiles [128, t_s*128]
  bf16; one-hot [128 pair-rows x 128 vslots] built by iota==vin (batched k
  tiles per DVE instruction, an act_frac share on ScalarE via
  relu(1-(iota-v)^2)); per tile one matmul accumulates onehot.T @ raw pairs
  into PSUM [128, (f,q)=128] f32; DVE adds the two q columns per f (one
  reduce per slot), ScalarE multiplies by the resident 1/degree column and
  writes a bf16 group output tile, DMA'd out once per 7-slot group.
"""

import sys

if "/opt/trn_rl_repo" not in sys.path:
    sys.path.insert(0, "/opt/trn_rl_repo")

import ml_dtypes
import numpy as np

# Problem constants (hardcoded per contract)
B, V, T, N, F, M = 1, 100000, 2, 32, 64, 1600000
NCORES = 8
BLK = 128           # pair-rows per tile (matmul contraction)
BLK_V = 128         # vertices per block / one-hot width
Q = 2               # edges pre-summed per pair-row
ROW_W = Q * F       # bf16 words per pair-row (128)
G = 7               # slots per output group
VLOC = V // NCORES          # 12500
NBLK = -(-VLOC // BLK_V)    # 98
NGRP = -(-NBLK // G)        # 14
VPAD = NBLK * BLK_V         # 12544


class Cfg:
    def __init__(self):
        self.V = V
        self.NCORES = NCORES
        self.VLOC = VLOC
        self.NBLK = NBLK
        self.VPAD = VPAD


_DEFAULT_CFG = Cfg()
_NC_CACHE = {}


def build_device_program(
    profile, cfg=_DEFAULT_CFG, act_frac=0.10, b_frac=0.85, oh_chunk=8, ring_mod=6
):
    """Build + compile the per-core Bass program.

    One-hot builds are batched k tiles per DVE instruction in two layouts:
    'a' = (i v) broadcast-last (slower DVE, contiguous LDWEIGHTS) and
    'b' = (v i) all-packed transposed (faster DVE, strided LDWEIGHTS);
    b_frac balances DVE vs PE load.  act_frac of tiles go to ScalarE via
    relu(1-(iota-v)^2).  Every ring_mod'th feature DMA rides the Activation
    HWDGE ring for extra DMA bandwidth."""
    from contextlib import ExitStack

    import concourse.tile as tile
    from concourse import bacc, mybir

    f32 = mybir.dt.float32
    bf16 = mybir.dt.bfloat16
    assert len(profile) == NBLK
    prof = np.asarray(profile, dtype=np.int64)
    t_max = int(prof.max())
    TT = int(prof.sum())                      # total tiles per table
    # slot-PAIR interleaved stream: each partition's two slot segments are
    # contiguous, so one 2D DMA covers a pair of any widths
    wpair = (prof[0::2] + prof[1::2]) * ROW_W          # words per partition
    peb = np.concatenate([[0], np.cumsum(BLK * wpair)]).astype(np.int64)
    tb = np.concatenate([[0], np.cumsum(prof)]).astype(np.int64)
    total_elems = int(peb[-1])

    nc = bacc.Bacc("TRN2", target_bir_lowering=False, debug=False)
    feat_d = [
        nc.dram_tensor(f"feat{t}", [total_elems], bf16, kind="ExternalInput").ap()
        for t in range(T)
    ]
    vin_d = [
        nc.dram_tensor(f"vin{t}", [BLK, TT], bf16, kind="ExternalInput").ap()
        for t in range(T)
    ]
    iota_d = nc.dram_tensor("iota", [BLK, BLK_V], bf16, kind="ExternalInput").ap()
    iotar_d = nc.dram_tensor(
        "iotar", [BLK, oh_chunk * BLK_V], bf16, kind="ExternalInput"
    ).ap()
    out_d = nc.dram_tensor(
        "out", [NGRP, BLK_V, G * T * F], bf16, kind="ExternalOutput"
    ).ap()

    with tile.TileContext(nc) as tc, ExitStack() as ctx:
        const = ctx.enter_context(tc.tile_pool(name="const", bufs=1))
        featp = ctx.enter_context(tc.tile_pool(name="featp", bufs=8))
        ohdp = ctx.enter_context(tc.tile_pool(name="ohdp", bufs=6))
        ohap = ctx.enter_context(tc.tile_pool(name="ohap", bufs=7))
        outp = ctx.enter_context(tc.tile_pool(name="outp", bufs=3))
        psump = ctx.enter_context(tc.tile_pool(name="psum", bufs=8, space="PSUM"))

        # Const loads ride the Activation engine's HWDGE queues so they can
        # never get stuck behind feature DMAs (sync queues) that wait on
        # tile-pool recycling.
        iota_t = const.tile([BLK, BLK_V], bf16)
        nc.scalar.dma_start(out=iota_t[:], in_=iota_d[:])
        iotar_t = const.tile([BLK, oh_chunk * BLK_V], bf16)
        nc.scalar.dma_start(out=iotar_t[:], in_=iotar_d[:])
        vin_t = []
        vinf_t = []
        for t in range(T):
            vt = const.tile([BLK, TT], bf16, tag=f"vin{t}")
            # 4-chunk column split so the load spreads across DMA queues
            bnds = [TT * i // 4 for i in range(5)]
            for a, b in zip(bnds[:-1], bnds[1:]):
                if b > a:
                    nc.scalar.dma_start(out=vt[:, a:b], in_=vin_d[t][:, a:b])
            vin_t.append(vt)
            # f32 negated copy for the ScalarE activation-bias one-hot path
            vf = const.tile([BLK, TT], f32, tag=f"vinf{t}")
            nc.vector.tensor_scalar(
                vf[:], vt[:], -1.0, None, op0=mybir.AluOpType.mult
            )
            vinf_t.append(vf)

        # --- one-hot build planning: weighted chunk assignment
        n_act = [0]
        n_b = [0]
        n_tot = [0]

        def plan_chunks(t_s):
            chunks = []
            i = 0
            while i < t_s:
                rest = t_s - i
                if n_act[0] < act_frac * n_tot[0]:
                    k = 1
                    chunks.append(("act", i, k))
                    n_act[0] += 1
                elif rest >= oh_chunk and n_b[0] < b_frac * n_tot[0]:
                    k = oh_chunk
                    chunks.append(("b", i, k))
                    n_b[0] += k
                else:
                    k = min(oh_chunk, rest)
                    chunks.append(("a", i, k))
                i += k
                n_tot[0] += k
            return chunks

        ndma = [0]
        out_tiles = {}

        def compute_slot(s, t, feat_t, off):
            """One (slot, table) pass: one-hots, matmuls, pair-reduce."""
            t_s = int(prof[s])
            g, so = divmod(s, G)
            if g not in out_tiles:
                ot = outp.tile([BLK_V, G * T * F], bf16, tag="outg")
                out_tiles[g] = ot
            out_t = out_tiles[g]
            refs = [None] * t_s
            for kind, i0, k in plan_chunks(t_s):
                c0 = int(tb[s]) + i0
                if kind == "act":
                    bias = vinf_t[t][:, c0 : c0 + 1]
                    y = ohap.tile([BLK, BLK_V], bf16, tag="y")
                    nc.scalar.activation(
                        y[:], iota_t[:],
                        mybir.ActivationFunctionType.Square,
                        bias=bias, scale=1.0,
                    )
                    oh = ohap.tile([BLK, BLK_V], bf16, tag="oha")
                    nc.scalar.activation(
                        oh[:], y[:], mybir.ActivationFunctionType.Relu,
                        bias=1.0, scale=-1.0,
                    )
                    refs[i0] = (oh, 0, 1)
                elif kind == "b":
                    oh = ohdp.tile([BLK, oh_chunk * BLK_V], bf16, tag="ohb")
                    oh3 = oh[:, : k * BLK_V].rearrange("p (v i) -> p v i", i=k)
                    in0 = iotar_t[:].rearrange("p (v i) -> p v i", i=k)
                    in1 = (
                        vin_t[t][:, c0 : c0 + k]
                        .unsqueeze(1)
                        .broadcast_to([BLK, BLK_V, k])
                    )
                    nc.vector.tensor_tensor(
                        oh3, in0, in1, op=mybir.AluOpType.is_equal
                    )
                    for j in range(k):
                        refs[i0 + j] = (oh, j, k)
                else:
                    oh = ohdp.tile([BLK, oh_chunk * BLK_V], bf16, tag="oha2")
                    oh3 = oh[:, : k * BLK_V].rearrange("p (i v) -> p i v", v=BLK_V)
                    in0 = iota_t[:].unsqueeze(1).broadcast_to([BLK, k, BLK_V])
                    in1 = (
                        vin_t[t][:, c0 : c0 + k]
                        .unsqueeze(2)
                        .broadcast_to([BLK, k, BLK_V])
                    )
                    nc.vector.tensor_tensor(
                        oh3, in0, in1, op=mybir.AluOpType.is_equal
                    )
                    for j in range(k):
                        refs[i0 + j] = (oh, j * BLK_V, 1)
            ps = psump.tile([BLK_V, ROW_W], f32)
            for i in range(t_s):
                oh, idx, stride = refs[i]
                if stride == 1:
                    lhsT = oh[:, idx : idx + BLK_V]
                else:
                    lhsT = oh[:, : stride * BLK_V].rearrange(
                        "p (v i) -> p i v", i=stride
                    )[:, idx, :]
                nc.tensor.matmul(
                    ps[:],
                    lhsT=lhsT,
                    rhs=feat_t[:, (off + i) * ROW_W : (off + i + 1) * ROW_W],
                    start=(i == 0),
                    stop=(i == t_s - 1),
                )
            # pair post-add: reduce psum (q f) over q -> bf16 out column
            with nc.allow_low_precision(reason="bf16 mean output"):
                nc.vector.tensor_reduce(
                    out_t[:, (so * T + t) * F : (so * T + t + 1) * F],
                    ps[:].rearrange("p (q f) -> p f q", q=Q),
                    axis=mybir.AxisListType.X,
                    op=mybir.AluOpType.add,
                )

        for s0 in range(0, NBLK, 2):
            pid = s0 // 2
            w2 = int(wpair[pid])
            for t in range(T):
                feat_t = featp.tile([BLK, 2 * t_max * ROW_W], bf16, tag="feat")
                src = feat_d[t][int(peb[pid]) : int(peb[pid + 1])].rearrange(
                    "(e w) -> e w", w=w2
                )
                ndma[0] += 1
                deng = nc.scalar if ndma[0] % ring_mod == 0 else nc.sync
                deng.dma_start(out=feat_t[:, :w2], in_=src)
                compute_slot(s0, t, feat_t, 0)
                compute_slot(s0 + 1, t, feat_t, int(prof[s0]))
            for s in (s0, s0 + 1):
                if s % G == G - 1:
                    g = s // G
                    nc.scalar.dma_start(out=out_d[g], in_=out_tiles.pop(g)[:])

    nc.compile()
    return nc


def shard_table(indices, cfg=_DEFAULT_CFG):
    """Per-edge placement: sort by destination, pair same-dest edges.

    Returns (order, core, slot-free fields) needed by make_profile/fill:
      dict with per-edge (sorted order) arrays and per-(core,block) tile counts.
    """
    v = np.ascontiguousarray(indices[:, 1]).astype(np.int64)
    order = np.argsort(v, kind="stable")
    vs = v[order]
    # per-vertex counts and ranks
    n_v = np.bincount(vs, minlength=V)
    starts = np.concatenate([[0], np.cumsum(n_v)])
    r = np.arange(len(vs), dtype=np.int64) - starts[vs]
    pv = (n_v + 1) // 2                                  # pairs per vertex
    pb = np.concatenate([[0], np.cumsum(pv)])            # pair id base per vertex
    core = vs // VLOC
    vloc = vs % VLOC
    blk = vloc // BLK_V
    vin = vloc % BLK_V
    # pair-row rank within (core, block)
    bps = pb[(np.arange(NCORES)[:, None] * VLOC + np.arange(NBLK)[None, :] * BLK_V)]
    pr = (pb[vs] - bps[core, blk]) + r // 2
    # per (core, block) pair counts -> tiles
    pv_pad = np.zeros(NCORES * VPAD, dtype=np.int64)
    pv_pad_idx = np.arange(V)
    pv_pad[(pv_pad_idx // VLOC) * VPAD + (pv_pad_idx % VLOC)] = pv
    cb = pv_pad.reshape(NCORES, NBLK, BLK_V).sum(axis=2)
    tiles = -(-cb // BLK)                                # [NCORES, NBLK]
    return {
        "order": order, "core": core, "blk": blk, "vin": vin,
        "pr": pr, "q": (r & 1).astype(np.int64), "tiles": tiles,
    }


def make_profile(tables):
    """Shared slot tile profile + per (table, core) slot->block permutation."""
    perms = []
    sorted_tiles = []
    for tab in tables:
        perms_t = []
        for c in range(NCORES):
            tl = tab["tiles"][c]
            p = np.argsort(-tl, kind="stable")
            perms_t.append(p)
            sorted_tiles.append(tl[p])
        perms.append(np.stack(perms_t))
    profile = np.max(np.stack(sorted_tiles), axis=0)
    profile = np.maximum(profile, 1)
    return [int(x) for x in profile], perms


def fill_streams(tab, features, rec_e, profile, perm, cfg=_DEFAULT_CFG):
    """Per-core bf16 feature stream (pre-scaled by 1/degree) + vin stream.

    Row layout is (q f): word q*F + f, so the PSUM pair halves are the
    contiguous column blocks [0:F] and [F:2F]."""
    prof = np.asarray(profile, dtype=np.int64)
    TT = int(prof.sum())
    wpair = (prof[0::2] + prof[1::2]) * ROW_W
    peb = np.concatenate([[0], np.cumsum(BLK * wpair)]).astype(np.int64)
    tb = np.concatenate([[0], np.cumsum(prof)]).astype(np.int64)
    TW = int(peb[-1])
    soff = np.zeros(NBLK, dtype=np.int64)
    soff[1::2] = prof[0::2] * ROW_W

    inv = np.empty((NCORES, NBLK), dtype=np.int64)
    for c in range(NCORES):
        inv[c, perm[c]] = np.arange(NBLK)

    scaled = features[tab["order"]] * rec_e[:, None]
    hi_u = scaled.astype(ml_dtypes.bfloat16).view(np.uint16)

    core = tab["core"]
    s_e = inv[core, tab["blk"]]
    p = tab["pr"] & (BLK - 1)
    i = tab["pr"] >> 7
    q = tab["q"]
    pid_e = s_e >> 1
    pos = (
        core * TW + peb[pid_e] + p * wpair[pid_e] + soff[s_e] + i * ROW_W + q * F
    )
    stream = np.zeros(NCORES * TW, dtype=np.uint16)
    cols = np.arange(F, dtype=np.int64)[None, :]
    stream[pos[:, None] + cols] = hi_u
    stream = stream.reshape(NCORES, TW).view(ml_dtypes.bfloat16)

    # vin stream [NCORES, 128, TT]; padding rows get -1 (never matches iota)
    vin_arr = np.full(NCORES * BLK * TT, -1.0, dtype=ml_dtypes.bfloat16)
    m0 = q == 0
    flat = core[m0] * (BLK * TT) + p[m0] * TT + (tb[s_e[m0]] + i[m0])
    vin_arr[flat] = tab["vin"][m0].astype(ml_dtypes.bfloat16)
    vin_arr = vin_arr.reshape(NCORES, BLK, TT)
    return stream, vin_arr


def edge_recip(adjacency, tab, t):
    """1/degree at each sorted edge's destination for table t."""
    adj = np.asarray(adjacency).reshape(V, T, N)
    deg = np.maximum((adj[:, t] >= 0).sum(axis=-1), 1).astype(np.float64)  # [V]
    rec = (1.0 / deg).astype(np.float32)
    vs = (tab["core"] * VLOC + tab["blk"] * BLK_V + tab["vin"]).astype(np.int64)
    return rec[vs]


def prepare_inputs(adjacency, indices0, features0, indices1, features1, cfg=_DEFAULT_CFG, oh_chunk=8):
    tab0 = shard_table(np.asarray(indices0), cfg)
    tab1 = shard_table(np.asarray(indices1), cfg)
    profile, perms = make_profile([tab0, tab1])

    r0 = edge_recip(adjacency, tab0, 0)
    r1 = edge_recip(adjacency, tab1, 1)
    f0, v0 = fill_streams(tab0, np.asarray(features0, dtype=np.float32), r0, profile, perms[0], cfg)
    f1, v1 = fill_streams(tab1, np.asarray(features1, dtype=np.float32), r1, profile, perms[1], cfg)
    iota = np.broadcast_to(
        np.arange(BLK_V).astype(ml_dtypes.bfloat16), (BLK, BLK_V)
    ).copy()
    iotar = np.broadcast_to(
        (np.arange(oh_chunk * BLK_V) // oh_chunk).astype(ml_dtypes.bfloat16),
        (BLK, oh_chunk * BLK_V),
    ).copy()

    in_maps = [
        {
            "feat0": f0[c],
            "feat1": f1[c],
            "vin0": v0[c],
            "vin1": v1[c],
            "iota": iota,
            "iotar": iotar,
        }
        for c in range(NCORES)
    ]
    return in_maps, profile, perms


def assemble_output(core_outs, perms, cfg=_DEFAULT_CFG):
    outs = []
    for t in range(T):
        parts = []
        for c in range(NCORES):
            arr = np.asarray(core_outs[c]).astype(np.float32)
            arr = arr.reshape(NGRP, BLK_V, G, T, F)[:, :, :, t, :]
            arr = arr.transpose(0, 2, 1, 3).reshape(NGRP * G, BLK_V, F)[:NBLK]
            tmp = np.empty((NBLK, BLK_V, F), dtype=np.float32)
            tmp[perms[t][c]] = arr
            parts.append(tmp.reshape(VPAD, F)[:VLOC])
        outs.append(np.concatenate(parts, axis=0).reshape(B, V, F))
    return (outs[0], outs[1])


def kernel(adjacency, indices0, features0, indices1, features1):
    from concourse.bass_utils import run_bass_kernel_spmd

    cfg = _DEFAULT_CFG
    in_maps, profile, perms = prepare_inputs(
        adjacency, indices0, features0, indices1, features1, cfg
    )

    key = tuple(profile)
    if key not in _NC_CACHE:
        _NC_CACHE[key] = build_device_program(profile, cfg)
    nc = _NC_CACHE[key]

    res = run_bass_kernel_spmd(nc, in_maps, list(range(NCORES)))
    return assemble_output(
        [res.results[c]["out"] for c in range(NCORES)], perms, cfg
    )


# revision 16
# speedup vs baseline: 2.3637x; 1.0227x over previous
"""Trainium2 Bass kernel for GNN mean aggregation (nn_AggrGSMean).

Computes, for t in {0,1}:
    out_t[b, v, :] = segment_sum(features_t over edges with dest v) / degree[b, v, t]
where degree[b, v, t] = max(count(adjacency[b, v, t, :] >= 0), 1).

Strategy (graph-partition sharding per the problem's sharding hint):
- Host: partition edges by destination-vertex range across 8 cores, sort each
  core's edges by destination.  Edges of the same destination are PAIRED
  (Q=2, odd counts padded with a zero edge); each pair-row carries the two
  edges' features interleaved feature-major (f0e0 f0e1 f1e0 ...) in bf16.
  Pair-rows are grouped into 128-vertex blocks; each block's pair list is
  padded to whole 128-row tiles.  Blocks are slot-assigned in decreasing
  tile-count order so one static per-slot profile (max over cores/tables)
  serves all cores.  The destination slot-vertex of each pair-row ships as a
  separate bf16 "vin" stream [128, total_tiles]; reciprocal degrees are
  computed on host and shipped as f32 [128, NBLK*T].
- Device (per core): per (slot, t): DMA the slot's pair tiles [128, t_s*128]
  bf16; one-hot [128 pair-rows x 128 vslots] built by iota==vin (batched k
  tiles per DVE instruction, an act_frac share on ScalarE via
  relu(1-(iota-v)^2)); per tile one matmul accumulates onehot.T @ raw pairs
  into PSUM [128, (f,q)=128] f32; DVE adds the two q columns per f (one
  reduce per slot), ScalarE multiplies by the resident 1/degree column and
  writes a bf16 group output tile, DMA'd out once per 7-slot group.
"""

import sys

if "/opt/trn_rl_repo" not in sys.path:
    sys.path.insert(0, "/opt/trn_rl_repo")

import ml_dtypes
import numpy as np

# Problem constants (hardcoded per contract)
B, V, T, N, F, M = 1, 100000, 2, 32, 64, 1600000
NCORES = 8
BLK = 128           # pair-rows per tile (matmul contraction)
BLK_V = 128         # vertices per block / one-hot width
Q = 2               # edges pre-summed per pair-row
ROW_W = Q * F       # bf16 words per pair-row (128)
G = 7               # slots per output group
VLOC = V // NCORES          # 12500
NBLK = -(-VLOC // BLK_V)    # 98
NGRP = -(-NBLK // G)        # 14
VPAD = NBLK * BLK_V         # 12544


class Cfg:
    def __init__(self):
        self.V = V
        self.NCORES = NCORES
        self.VLOC = VLOC
        self.NBLK = NBLK
        self.VPAD = VPAD


_DEFAULT_CFG = Cfg()
_NC_CACHE = {}


def build_device_program(
    profile, cfg=_DEFAULT_CFG, act_frac=0.06, oh_chunk=8, gp_mod=12
):
    """Build + compile the per-core Bass program.

    One-hot builds are batched k tiles per DVE instruction in two layouts:
    'a' = (i v) broadcast-last (slower DVE, contiguous LDWEIGHTS) and
    'b' = (v i) all-packed transposed (faster DVE, strided LDWEIGHTS);
    b_frac balances DVE vs PE load.  act_frac of tiles go to ScalarE via
    relu(1-(iota-v)^2).  Every ring_mod'th feature DMA rides the Activation
    HWDGE ring for extra DMA bandwidth."""
    from contextlib import ExitStack

    import concourse.tile as tile
    from concourse import bacc, mybir

    f32 = mybir.dt.float32
    bf16 = mybir.dt.bfloat16
    assert len(profile) == NBLK
    prof = np.asarray(profile, dtype=np.int64)
    t_max = int(prof.max())
    TT = int(prof.sum())                      # total tiles per table
    # slot-PAIR interleaved stream: each partition's two slot segments are
    # contiguous, so one 2D DMA covers a pair of any widths
    wpair = (prof[0::2] + prof[1::2]) * ROW_W          # words per partition
    peb = np.concatenate([[0], np.cumsum(BLK * wpair)]).astype(np.int64)
    tb = np.concatenate([[0], np.cumsum(prof)]).astype(np.int64)
    total_elems = int(peb[-1])

    nc = bacc.Bacc("TRN2", target_bir_lowering=False, debug=False)
    feat_d = [
        nc.dram_tensor(f"feat{t}", [total_elems], bf16, kind="ExternalInput").ap()
        for t in range(T)
    ]
    vin_d = [
        nc.dram_tensor(f"vin{t}", [BLK, TT], bf16, kind="ExternalInput").ap()
        for t in range(T)
    ]
    iota_d = nc.dram_tensor("iota", [BLK, BLK_V], bf16, kind="ExternalInput").ap()
    iotar_d = nc.dram_tensor(
        "iotar", [BLK, oh_chunk * BLK_V], bf16, kind="ExternalInput"
    ).ap()
    out_d = nc.dram_tensor(
        "out", [NGRP, BLK_V, G * T * F], bf16, kind="ExternalOutput"
    ).ap()

    with tile.TileContext(nc) as tc, ExitStack() as ctx:
        const = ctx.enter_context(tc.tile_pool(name="const", bufs=1))
        featp = ctx.enter_context(tc.tile_pool(name="featp", bufs=8))
        ohdp = ctx.enter_context(tc.tile_pool(name="ohdp", bufs=6))
        ohap = ctx.enter_context(tc.tile_pool(name="ohap", bufs=7))
        outp = ctx.enter_context(tc.tile_pool(name="outp", bufs=3))
        psump = ctx.enter_context(tc.tile_pool(name="psum", bufs=8, space="PSUM"))

        # Const loads ride the Activation engine's HWDGE queues so they can
        # never get stuck behind feature DMAs (sync queues) that wait on
        # tile-pool recycling.
        iota_t = const.tile([BLK, BLK_V], bf16)
        nc.scalar.dma_start(out=iota_t[:], in_=iota_d[:])
        iotar_t = const.tile([BLK, oh_chunk * BLK_V], bf16)
        nc.scalar.dma_start(out=iotar_t[:], in_=iotar_d[:])
        vin_t = []
        vinf_t = []
        for t in range(T):
            vt = const.tile([BLK, TT], bf16, tag=f"vin{t}")
            # 4-chunk column split so the load spreads across DMA queues
            bnds = [TT * i // 4 for i in range(5)]
            for a, b in zip(bnds[:-1], bnds[1:]):
                if b > a:
                    nc.scalar.dma_start(out=vt[:, a:b], in_=vin_d[t][:, a:b])
            vin_t.append(vt)
            # f32 negated copy for the ScalarE activation-bias one-hot path
            vf = const.tile([BLK, TT], f32, tag=f"vinf{t}")
            nc.vector.tensor_scalar(
                vf[:], vt[:], -1.0, None, op0=mybir.AluOpType.mult
            )
            vinf_t.append(vf)

        # --- one-hot build planning: weighted chunk assignment
        n_act = [0]
        n_tot = [0]

        def plan_chunks(t_s):
            chunks = []
            i = 0
            while i < t_s:
                rest = t_s - i
                if n_act[0] < act_frac * n_tot[0]:
                    k = 1
                    chunks.append(("act", i, k))
                    n_act[0] += 1
                else:
                    k = min(oh_chunk, rest)
                    chunks.append(("b", i, k))
                i += k
                n_tot[0] += k
            return chunks

        ndma = [0]
        out_tiles = {}

        def compute_slot(s, t, feat_t, off, ps, half):
            """One (slot, table) pass: one-hots + matmuls into psum half."""
            t_s = int(prof[s])
            g, so = divmod(s, G)
            if g not in out_tiles:
                ot = outp.tile([BLK_V, G * T * F], bf16, tag="outg")
                out_tiles[g] = ot
            refs = [None] * t_s
            for kind, i0, k in plan_chunks(t_s):
                c0 = int(tb[s]) + i0
                if kind == "act":
                    bias = vinf_t[t][:, c0 : c0 + 1]
                    y = ohap.tile([BLK, BLK_V], bf16, tag="y")
                    nc.scalar.activation(
                        y[:], iota_t[:],
                        mybir.ActivationFunctionType.Square,
                        bias=bias, scale=1.0,
                    )
                    oh = ohap.tile([BLK, BLK_V], bf16, tag="oha")
                    nc.scalar.activation(
                        oh[:], y[:], mybir.ActivationFunctionType.Relu,
                        bias=1.0, scale=-1.0,
                    )
                    refs[i0] = (oh, 0, 1)
                else:
                    oh = ohdp.tile([BLK, oh_chunk * BLK_V], bf16, tag="ohb")
                    oh3 = oh[:, : k * BLK_V].rearrange("p (v i) -> p v i", i=k)
                    in0 = iotar_t[:].rearrange(
                        "p (v i) -> p v i", i=oh_chunk
                    )[:, :, :k]
                    in1 = (
                        vin_t[t][:, c0 : c0 + k]
                        .unsqueeze(1)
                        .broadcast_to([BLK, BLK_V, k])
                    )
                    nc.vector.tensor_tensor(
                        oh3, in0, in1, op=mybir.AluOpType.is_equal
                    )
                    for j in range(k):
                        refs[i0 + j] = (oh, j, k)
            for i in range(t_s):
                oh, idx, stride = refs[i]
                if stride == 1:
                    lhsT = oh[:, idx : idx + BLK_V]
                else:
                    lhsT = oh[:, : stride * BLK_V].rearrange(
                        "p (v i) -> p i v", i=stride
                    )[:, idx, :]
                nc.tensor.matmul(
                    ps[:, half * ROW_W : (half + 1) * ROW_W],
                    lhsT=lhsT,
                    rhs=feat_t[:, (off + i) * ROW_W : (off + i + 1) * ROW_W],
                    start=(i == 0),
                    stop=(i == t_s - 1),
                )

        for s0 in range(0, NBLK, 2):
            pid = s0 // 2
            w2 = int(wpair[pid])
            g0, so0 = divmod(s0, G)
            cross = (s0 % G) == G - 1
            for t in range(T):
                feat_t = featp.tile([BLK, 2 * t_max * ROW_W], bf16, tag="feat")
                src = feat_d[t][int(peb[pid]) : int(peb[pid + 1])].rearrange(
                    "(e w) -> e w", w=w2
                )
                ndma[0] += 1
                if ndma[0] % gp_mod == 0:
                    deng = nc.gpsimd
                elif ndma[0] % 2 == 0:
                    deng = nc.scalar
                else:
                    deng = nc.sync
                deng.dma_start(out=feat_t[:, :w2], in_=src)
                ps = psump.tile([BLK_V, 2 * ROW_W], f32)
                compute_slot(s0, t, feat_t, 0, ps, 0)
                compute_slot(s0 + 1, t, feat_t, int(prof[s0]), ps, 1)
                with nc.allow_low_precision(reason="bf16 mean output"):
                    if not cross:
                        vv = out_tiles[g0][:].rearrange(
                            "p (so tt f) -> p so tt f", so=G, tt=T
                        )
                        nc.vector.tensor_reduce(
                            vv[:, so0 : so0 + 2, t, :],
                            ps[:].rearrange(
                                "p (s2 q f) -> p s2 f q", s2=2, q=Q
                            ),
                            axis=mybir.AxisListType.X,
                            op=mybir.AluOpType.add,
                        )
                    else:
                        for h, s in ((0, s0), (1, s0 + 1)):
                            gh, soh = divmod(s, G)
                            nc.vector.tensor_reduce(
                                out_tiles[gh][
                                    :, (soh * T + t) * F : (soh * T + t + 1) * F
                                ],
                                ps[:, h * ROW_W : (h + 1) * ROW_W].rearrange(
                                    "p (q f) -> p f q", q=Q
                                ),
                                axis=mybir.AxisListType.X,
                                op=mybir.AluOpType.add,
                            )
            for s in (s0, s0 + 1):
                if s % G == G - 1:
                    g = s // G
                    nc.sync.dma_start(out=out_d[g], in_=out_tiles.pop(g)[:])

    nc.compile()
    return nc


def shard_table(indices, cfg=_DEFAULT_CFG):
    """Per-edge placement: sort by destination, pair same-dest edges.

    Returns (order, core, slot-free fields) needed by make_profile/fill:
      dict with per-edge (sorted order) arrays and per-(core,block) tile counts.
    """
    v = np.ascontiguousarray(indices[:, 1]).astype(np.int64)
    order = np.argsort(v, kind="stable")
    vs = v[order]
    # per-vertex counts and ranks
    n_v = np.bincount(vs, minlength=V)
    starts = np.concatenate([[0], np.cumsum(n_v)])
    r = np.arange(len(vs), dtype=np.int64) - starts[vs]
    pv = (n_v + 1) // 2                                  # pairs per vertex
    pb = np.concatenate([[0], np.cumsum(pv)])            # pair id base per vertex
    core = vs // VLOC
    vloc = vs % VLOC
    blk = vloc // BLK_V
    vin = vloc % BLK_V
    # pair-row rank within (core, block)
    bps = pb[(np.arange(NCORES)[:, None] * VLOC + np.arange(NBLK)[None, :] * BLK_V)]
    pr = (pb[vs] - bps[core, blk]) + r // 2
    # per (core, block) pair counts -> tiles
    pv_pad = np.zeros(NCORES * VPAD, dtype=np.int64)
    pv_pad_idx = np.arange(V)
    pv_pad[(pv_pad_idx // VLOC) * VPAD + (pv_pad_idx % VLOC)] = pv
    cb = pv_pad.reshape(NCORES, NBLK, BLK_V).sum(axis=2)
    tiles = -(-cb // BLK)                                # [NCORES, NBLK]
    return {
        "order": order, "core": core, "blk": blk, "vin": vin,
        "pr": pr, "q": (r & 1).astype(np.int64), "tiles": tiles,
    }


def make_profile(tables):
    """Shared slot tile profile + per (table, core) slot->block permutation."""
    perms = []
    sorted_tiles = []
    for tab in tables:
        perms_t = []
        for c in range(NCORES):
            tl = tab["tiles"][c]
            p = np.argsort(-tl, kind="stable")
            perms_t.append(p)
            sorted_tiles.append(tl[p])
        perms.append(np.stack(perms_t))
    profile = np.max(np.stack(sorted_tiles), axis=0)
    profile = np.maximum(profile, 1)
    return [int(x) for x in profile], perms


def fill_streams(tab, features, rec_e, profile, perm, cfg=_DEFAULT_CFG):
    """Per-core bf16 feature stream (pre-scaled by 1/degree) + vin stream.

    Row layout is (q f): word q*F + f, so the PSUM pair halves are the
    contiguous column blocks [0:F] and [F:2F]."""
    prof = np.asarray(profile, dtype=np.int64)
    TT = int(prof.sum())
    wpair = (prof[0::2] + prof[1::2]) * ROW_W
    peb = np.concatenate([[0], np.cumsum(BLK * wpair)]).astype(np.int64)
    tb = np.concatenate([[0], np.cumsum(prof)]).astype(np.int64)
    TW = int(peb[-1])
    soff = np.zeros(NBLK, dtype=np.int64)
    soff[1::2] = prof[0::2] * ROW_W

    inv = np.empty((NCORES, NBLK), dtype=np.int64)
    for c in range(NCORES):
        inv[c, perm[c]] = np.arange(NBLK)

    scaled = features[tab["order"]] * rec_e[:, None]
    hi_u = scaled.astype(ml_dtypes.bfloat16).view(np.uint16)

    core = tab["core"]
    s_e = inv[core, tab["blk"]]
    p = tab["pr"] & (BLK - 1)
    i = tab["pr"] >> 7
    q = tab["q"]
    pid_e = s_e >> 1
    pos = (
        core * TW + peb[pid_e] + p * wpair[pid_e] + soff[s_e] + i * ROW_W + q * F
    )
    stream = np.zeros(NCORES * TW, dtype=np.uint16)
    cols = np.arange(F, dtype=np.int64)[None, :]
    stream[pos[:, None] + cols] = hi_u
    stream = stream.reshape(NCORES, TW).view(ml_dtypes.bfloat16)

    # vin stream [NCORES, 128, TT]; padding rows get -1 (never matches iota)
    vin_arr = np.full(NCORES * BLK * TT, -1.0, dtype=ml_dtypes.bfloat16)
    m0 = q == 0
    flat = core[m0] * (BLK * TT) + p[m0] * TT + (tb[s_e[m0]] + i[m0])
    vin_arr[flat] = tab["vin"][m0].astype(ml_dtypes.bfloat16)
    vin_arr = vin_arr.reshape(NCORES, BLK, TT)
    return stream, vin_arr


def edge_recip(adjacency, tab, t):
    """1/degree at each sorted edge's destination for table t."""
    adj = np.asarray(adjacency).reshape(V, T, N)
    deg = np.maximum((adj[:, t] >= 0).sum(axis=-1), 1).astype(np.float64)  # [V]
    rec = (1.0 / deg).astype(np.float32)
    vs = (tab["core"] * VLOC + tab["blk"] * BLK_V + tab["vin"]).astype(np.int64)
    return rec[vs]


def prepare_inputs(adjacency, indices0, features0, indices1, features1, cfg=_DEFAULT_CFG, oh_chunk=8):
    tab0 = shard_table(np.asarray(indices0), cfg)
    tab1 = shard_table(np.asarray(indices1), cfg)
    profile, perms = make_profile([tab0, tab1])

    r0 = edge_recip(adjacency, tab0, 0)
    r1 = edge_recip(adjacency, tab1, 1)
    f0, v0 = fill_streams(tab0, np.asarray(features0, dtype=np.float32), r0, profile, perms[0], cfg)
    f1, v1 = fill_streams(tab1, np.asarray(features1, dtype=np.float32), r1, profile, perms[1], cfg)
    iota = np.broadcast_to(
        np.arange(BLK_V).astype(ml_dtypes.bfloat16), (BLK, BLK_V)
    ).copy()
    iotar = np.broadcast_to(
        (np.arange(oh_chunk * BLK_V) // oh_chunk).astype(ml_dtypes.bfloat16),
        (BLK, oh_chunk * BLK_V),
    ).copy()

    in_maps = [
        {
            "feat0": f0[c],
            "feat1": f1[c],
            "vin0": v0[c],
            "vin1": v1[c],
            "iota": iota,
            "iotar": iotar,
        }
        for c in range(NCORES)
    ]
    return in_maps, profile, perms


def assemble_output(core_outs, perms, cfg=_DEFAULT_CFG):
    outs = []
    for t in range(T):
        parts = []
        for c in range(NCORES):
            arr = np.asarray(core_outs[c]).astype(np.float32)
            arr = arr.reshape(NGRP, BLK_V, G, T, F)[:, :, :, t, :]
            arr = arr.transpose(0, 2, 1, 3).reshape(NGRP * G, BLK_V, F)[:NBLK]
            tmp = np.empty((NBLK, BLK_V, F), dtype=np.float32)
            tmp[perms[t][c]] = arr
            parts.append(tmp.reshape(VPAD, F)[:VLOC])
        outs.append(np.concatenate(parts, axis=0).reshape(B, V, F))
    return (outs[0], outs[1])


def kernel(adjacency, indices0, features0, indices1, features1):
    from concourse.bass_utils import run_bass_kernel_spmd

    cfg = _DEFAULT_CFG
    in_maps, profile, perms = prepare_inputs(
        adjacency, indices0, features0, indices1, features1, cfg
    )

    key = tuple(profile)
    if key not in _NC_CACHE:
        _NC_CACHE[key] = build_device_program(profile, cfg)
    nc = _NC_CACHE[key]

    res = run_bass_kernel_spmd(nc, in_maps, list(range(NCORES)))
    return assemble_output(
        [res.results[c]["out"] for c in range(NCORES)], perms, cfg
    )


# revision 24
# speedup vs baseline: 2.5882x; 1.0950x over previous
"""Trainium2 Bass kernel for GNN mean aggregation (nn_AggrGSMean).

Computes, for t in {0,1}:
    out_t[b, v, :] = segment_sum(features_t over edges with dest v) / degree[b, v, t]
where degree[b, v, t] = max(count(adjacency[b, v, t, :] >= 0), 1).

Strategy (graph-partition sharding per the problem's sharding hint):
- Host: partition edges by destination-vertex range across 8 cores, sort each
  core's edges by destination.  Edges of the same destination are PAIRED
  (Q=2, odd counts padded with a zero edge); each pair-row carries the two
  edges' features interleaved feature-major (f0e0 f0e1 f1e0 ...) in bf16.
  Pair-rows are grouped into 128-vertex blocks; each block's pair list is
  padded to whole 128-row tiles.  Blocks are slot-assigned in decreasing
  tile-count order so one static per-slot profile (max over cores/tables)
  serves all cores.  The destination slot-vertex of each pair-row ships as a
  separate bf16 "vin" stream [128, total_tiles]; reciprocal degrees are
  computed on host and shipped as f32 [128, NBLK*T].
- Device (per core): per (slot, t): DMA the slot's pair tiles [128, t_s*128]
  bf16; one-hot [128 pair-rows x 128 vslots] built by iota==vin (batched k
  tiles per DVE instruction, an act_frac share on ScalarE via
  relu(1-(iota-v)^2)); per tile one matmul accumulates onehot.T @ raw pairs
  into PSUM [128, (f,q)=128] f32; DVE adds the two q columns per f (one
  reduce per slot), ScalarE multiplies by the resident 1/degree column and
  writes a bf16 group output tile, DMA'd out once per 7-slot group.
"""

import sys

if "/opt/trn_rl_repo" not in sys.path:
    sys.path.insert(0, "/opt/trn_rl_repo")

import ml_dtypes
import numpy as np

# Problem constants (hardcoded per contract)
B, V, T, N, F, M = 1, 100000, 2, 32, 64, 1600000
NCORES = 8
BLK = 128           # pair-rows per tile (matmul contraction)
BLK_V = 128         # vertices per block / one-hot width
Q = 2               # edges pre-summed per pair-row
ROW_W = Q * F       # bf16 words per pair-row (128)
G = 7               # slots per output group
VLOC = V // NCORES          # 12500
NBLK = -(-VLOC // BLK_V)    # 98
NGRP = -(-NBLK // G)        # 14
VPAD = NBLK * BLK_V         # 12544


class Cfg:
    def __init__(self):
        self.V = V
        self.NCORES = NCORES
        self.VLOC = VLOC
        self.NBLK = NBLK
        self.VPAD = VPAD


_DEFAULT_CFG = Cfg()
_NC_CACHE = {}


def build_device_program(
    profile, cfg=_DEFAULT_CFG, act_frac=0.06, oh_chunk=8, gp_mod=12
):
    """Build + compile the per-core Bass program.

    One-hot builds are batched k tiles per DVE instruction in two layouts:
    'a' = (i v) broadcast-last (slower DVE, contiguous LDWEIGHTS) and
    'b' = (v i) all-packed transposed (faster DVE, strided LDWEIGHTS);
    b_frac balances DVE vs PE load.  act_frac of tiles go to ScalarE via
    relu(1-(iota-v)^2).  Every ring_mod'th feature DMA rides the Activation
    HWDGE ring for extra DMA bandwidth."""
    from contextlib import ExitStack

    import concourse.tile as tile
    from concourse import bacc, mybir

    f32 = mybir.dt.float32
    bf16 = mybir.dt.bfloat16
    assert len(profile) == NBLK
    prof = np.asarray(profile, dtype=np.int64)
    t_max = int(prof.max())
    TT = int(prof.sum())                      # total tiles per table
    # slot-PAIR interleaved stream: each partition's two slot segments are
    # contiguous, so one 2D DMA covers a pair of any widths
    wpair = (prof[0::2] + prof[1::2]) * ROW_W          # words per partition
    peb = np.concatenate([[0], np.cumsum(BLK * wpair)]).astype(np.int64)
    tb = np.concatenate([[0], np.cumsum(prof)]).astype(np.int64)
    total_elems = int(peb[-1])

    nc = bacc.Bacc("TRN2", target_bir_lowering=False, debug=False)
    feat_d = [
        nc.dram_tensor(f"feat{t}", [total_elems], bf16, kind="ExternalInput").ap()
        for t in range(T)
    ]
    vin_d = [
        nc.dram_tensor(f"vin{t}", [BLK, TT], bf16, kind="ExternalInput").ap()
        for t in range(T)
    ]
    iota_d = nc.dram_tensor("iota", [BLK, BLK_V], bf16, kind="ExternalInput").ap()
    iotar_d = nc.dram_tensor(
        "iotar", [BLK, oh_chunk * BLK_V], bf16, kind="ExternalInput"
    ).ap()
    out_d = nc.dram_tensor(
        "out", [NGRP, BLK_V, G * T * F], bf16, kind="ExternalOutput"
    ).ap()

    with tile.TileContext(nc) as tc, ExitStack() as ctx:
        const = ctx.enter_context(tc.tile_pool(name="const", bufs=1))
        featp = ctx.enter_context(tc.tile_pool(name="featp", bufs=8))
        ohdp = ctx.enter_context(tc.tile_pool(name="ohdp", bufs=6))
        ohap = ctx.enter_context(tc.tile_pool(name="ohap", bufs=7))
        outp = ctx.enter_context(tc.tile_pool(name="outp", bufs=3))
        psump = ctx.enter_context(tc.tile_pool(name="psum", bufs=8, space="PSUM"))

        # Const loads ride the Activation engine's HWDGE queues so they can
        # never get stuck behind feature DMAs (sync queues) that wait on
        # tile-pool recycling.
        iota_t = const.tile([BLK, BLK_V], bf16)
        nc.scalar.dma_start(out=iota_t[:], in_=iota_d[:])
        iotar_t = const.tile([BLK, oh_chunk * BLK_V], bf16)
        nc.scalar.dma_start(out=iotar_t[:], in_=iotar_d[:])
        vin_t = []
        vinf_t = []
        for t in range(T):
            vt = const.tile([BLK, TT], bf16, tag=f"vin{t}")
            # 4-chunk column split so the load spreads across DMA queues
            bnds = [TT * i // 4 for i in range(5)]
            for a, b in zip(bnds[:-1], bnds[1:]):
                if b > a:
                    nc.scalar.dma_start(out=vt[:, a:b], in_=vin_d[t][:, a:b])
            vin_t.append(vt)
            # f32 negated copy for the ScalarE activation-bias one-hot path
            vf = const.tile([BLK, TT], f32, tag=f"vinf{t}")
            nc.vector.tensor_scalar(
                vf[:], vt[:], -1.0, None, op0=mybir.AluOpType.mult
            )
            vinf_t.append(vf)

        # --- one-hot build planning: odd slots give their last tile to
        # ScalarE so every DVE chunk keeps an even lhsT stride (aligned
        # LDWEIGHTS); remaining tiles batch in chunks of oh_chunk.
        def plan_chunks(t_s):
            chunks = []
            end = t_s
            if t_s % 2 == 1 and t_s > 1:
                chunks.append(("act", t_s - 1, 1))
                end = t_s - 1
            i = 0
            while i < end:
                k = min(oh_chunk, end - i)
                chunks.append(("b", i, k))
                i += k
            return chunks

        ndma = [0]
        out_tiles = {}

        def compute_slot(s, t, feat_t, off, ps, half):
            """One (slot, table) pass: one-hots + matmuls into psum half."""
            t_s = int(prof[s])
            g, so = divmod(s, G)
            if g not in out_tiles:
                ot = outp.tile([BLK_V, G * T * F], bf16, tag="outg")
                out_tiles[g] = ot
            refs = [None] * t_s
            for kind, i0, k in plan_chunks(t_s):
                c0 = int(tb[s]) + i0
                if kind == "act":
                    bias = vinf_t[t][:, c0 : c0 + 1]
                    y = ohap.tile([BLK, BLK_V], bf16, tag="y")
                    nc.scalar.activation(
                        y[:], iota_t[:],
                        mybir.ActivationFunctionType.Square,
                        bias=bias, scale=1.0,
                    )
                    oh = ohap.tile([BLK, BLK_V], bf16, tag="oha")
                    nc.scalar.activation(
                        oh[:], y[:], mybir.ActivationFunctionType.Relu,
                        bias=1.0, scale=-1.0,
                    )
                    refs[i0] = (oh, 0, 1)
                else:
                    oh = ohdp.tile([BLK, oh_chunk * BLK_V], bf16, tag="ohb")
                    oh3 = oh[:, : k * BLK_V].rearrange("p (v i) -> p v i", i=k)
                    in0 = iotar_t[:].rearrange(
                        "p (v i) -> p v i", i=oh_chunk
                    )[:, :, :k]
                    in1 = (
                        vin_t[t][:, c0 : c0 + k]
                        .unsqueeze(1)
                        .broadcast_to([BLK, BLK_V, k])
                    )
                    nc.vector.tensor_tensor(
                        oh3, in0, in1, op=mybir.AluOpType.is_equal
                    )
                    for j in range(k):
                        refs[i0 + j] = (oh, j, k)
            for i in range(t_s):
                oh, idx, stride = refs[i]
                if stride == 1:
                    lhsT = oh[:, idx : idx + BLK_V]
                else:
                    lhsT = oh[:, : stride * BLK_V].rearrange(
                        "p (v i) -> p i v", i=stride
                    )[:, idx, :]
                nc.tensor.matmul(
                    ps[:, half * ROW_W : (half + 1) * ROW_W],
                    lhsT=lhsT,
                    rhs=feat_t[:, (off + i) * ROW_W : (off + i + 1) * ROW_W],
                    start=(i == 0),
                    stop=(i == t_s - 1),
                )

        for s0 in range(0, NBLK, 2):
            pid = s0 // 2
            w2 = int(wpair[pid])
            g0, so0 = divmod(s0, G)
            cross = (s0 % G) == G - 1
            for t in range(T):
                feat_t = featp.tile([BLK, 2 * t_max * ROW_W], bf16, tag="feat")
                src = feat_d[t][int(peb[pid]) : int(peb[pid + 1])].rearrange(
                    "(e w) -> e w", w=w2
                )
                ndma[0] += 1
                if ndma[0] % gp_mod == 0:
                    deng = nc.gpsimd
                elif ndma[0] % 2 == 0:
                    deng = nc.scalar
                else:
                    deng = nc.sync
                deng.dma_start(out=feat_t[:, :w2], in_=src)
                ps = psump.tile([BLK_V, 2 * ROW_W], f32)
                compute_slot(s0, t, feat_t, 0, ps, 0)
                compute_slot(s0 + 1, t, feat_t, int(prof[s0]), ps, 1)
                with nc.allow_low_precision(reason="bf16 mean output"):
                    if not cross:
                        vv = out_tiles[g0][:].rearrange(
                            "p (so tt f) -> p so tt f", so=G, tt=T
                        )
                        nc.vector.tensor_reduce(
                            vv[:, so0 : so0 + 2, t, :],
                            ps[:].rearrange(
                                "p (s2 q f) -> p s2 f q", s2=2, q=Q
                            ),
                            axis=mybir.AxisListType.X,
                            op=mybir.AluOpType.add,
                        )
                    else:
                        for h, s in ((0, s0), (1, s0 + 1)):
                            gh, soh = divmod(s, G)
                            nc.vector.tensor_reduce(
                                out_tiles[gh][
                                    :, (soh * T + t) * F : (soh * T + t + 1) * F
                                ],
                                ps[:, h * ROW_W : (h + 1) * ROW_W].rearrange(
                                    "p (q f) -> p f q", q=Q
                                ),
                                axis=mybir.AxisListType.X,
                                op=mybir.AluOpType.add,
                            )
            for s in (s0, s0 + 1):
                if s % G == G - 1:
                    g = s // G
                    nc.sync.dma_start(out=out_d[g], in_=out_tiles.pop(g)[:])

    nc.compile()
    return nc


def _pack_core(pv, x_high):
    """Pack VPAD vertices (pair counts pv) into NBLK bins of exactly BLK_V.

    Serpentine-deal by descending pv (near-equal sums), then swap-repair so
    at most x_high bins exceed 8 tiles (cap 9).  Returns bins [NBLK, BLK_V]
    of vertex ids."""
    order = np.argsort(-pv, kind="stable")
    deal = order.reshape(BLK_V, NBLK).copy()
    deal[1::2] = deal[1::2, ::-1]
    bins = np.ascontiguousarray(deal.T)          # [NBLK, BLK_V]
    sums = pv[bins].sum(axis=1)
    lo_cap, hi_cap = 8 * BLK, 9 * BLK
    hi = set(np.argsort(-sums)[:x_high])
    room = {h: hi_cap - sums[h] for h in hi}
    for b in range(NBLK):
        if b in hi:
            continue
        need = sums[b] - lo_cap
        it = 0
        while need > 0 and it < 40:
            it += 1
            h = max(room, key=room.get)
            if room[h] <= 0:
                break
            pb_ = pv[bins[b]]
            ph_ = pv[bins[h]]
            iu = int(np.argmax(pb_))
            iw = int(np.argmin(ph_))
            gain = int(pb_[iu] - ph_[iw])
            if gain <= 0:
                break
            gain = min(gain, int(room[h]) + 0)
            u, w = bins[b, iu], bins[h, iw]
            if pv[u] - pv[w] > room[h]:
                # find a better-matched u: largest pv[u] with delta <= room
                cand = np.where(pb_ - pv[w] <= room[h])[0]
                if len(cand) == 0:
                    break
                iu = int(cand[np.argmax(pb_[cand])])
                u = bins[b, iu]
                gain = int(pv[u] - pv[w])
                if gain <= 0:
                    break
            bins[b, iu], bins[h, iw] = w, u
            sums[b] -= gain
            sums[h] += gain
            room[h] -= gain
            need -= gain
    return bins


def shard_table(indices, x_high=None, cfg=_DEFAULT_CFG):
    """Per-edge placement: sort by destination, pair same-dest edges, and
    bin-pack vertices into blocks so slot tile counts are (mostly) 8 or 9."""
    v = np.ascontiguousarray(indices[:, 1]).astype(np.int64)
    order = np.argsort(v, kind="stable")
    vs = v[order]
    n_v = np.bincount(vs, minlength=V)
    starts = np.concatenate([[0], np.cumsum(n_v)])
    r = np.arange(len(vs), dtype=np.int64) - starts[vs]
    pv = (n_v + 1) // 2                                  # pairs per vertex
    core = vs // VLOC
    vloc_e = vs % VLOC

    pv_pad = np.zeros((NCORES, VPAD), dtype=np.int64)
    pv_pad[:, :VLOC] = pv.reshape(NCORES, VLOC)
    if x_high is None:
        tp = pv_pad.sum(axis=1)
        x_high = int(np.ceil((tp.max() - NBLK * 8 * BLK) / BLK)) + 1
        x_high = max(0, min(NBLK, x_high))

    vblk = np.empty((NCORES, VPAD), dtype=np.int64)
    vvin = np.empty((NCORES, VPAD), dtype=np.int64)
    pbb = np.empty((NCORES, VPAD), dtype=np.int64)   # pair base within block
    cnt = np.empty((NCORES, NBLK), dtype=np.int64)
    for c in range(NCORES):
        bins = _pack_core(pv_pad[c], x_high)
        flat = bins.reshape(-1)                      # slot index -> vertex
        slot_of = np.empty(VPAD, dtype=np.int64)
        slot_of[flat] = np.arange(VPAD)
        vblk[c] = slot_of // BLK_V
        vvin[c] = slot_of % BLK_V
        pv_by_slot = pv_pad[c][flat]
        cum = np.cumsum(pv_by_slot) - pv_by_slot     # exclusive
        blk_start = cum.reshape(NBLK, BLK_V)[:, 0]
        pb_by_slot = cum - np.repeat(blk_start, BLK_V)
        pbb[c] = pb_by_slot[slot_of]
        cnt[c] = pv_by_slot.reshape(NBLK, BLK_V).sum(axis=1)

    blk_e = vblk[core, vloc_e]
    vin_e = vvin[core, vloc_e]
    pr = pbb[core, vloc_e] + r // 2
    tiles = np.maximum(-(-cnt // BLK), 1)            # [NCORES, NBLK]
    return {
        "order": order, "core": core, "blk": blk_e, "vin": vin_e,
        "pr": pr, "q": (r & 1).astype(np.int64), "tiles": tiles,
        "vglob": vs, "vblk": vblk, "vvin": vvin,
    }


def make_profile(tables):
    """Shared slot tile profile + per (table, core) slot->block permutation.

    Returns (profile, meta) where meta carries the permutations and the
    per-table vertex->(block, vin) maps needed for output assembly."""
    perms = []
    sorted_tiles = []
    for tab in tables:
        perms_t = []
        for c in range(NCORES):
            tl = tab["tiles"][c]
            p = np.argsort(-tl, kind="stable")
            perms_t.append(p)
            sorted_tiles.append(tl[p])
        perms.append(np.stack(perms_t))
    profile = np.max(np.stack(sorted_tiles), axis=0)
    profile = np.maximum(profile, 1)
    meta = {
        "perm": perms,
        "vmap": [(tab["vblk"], tab["vvin"]) for tab in tables],
    }
    return [int(x) for x in profile], meta


def fill_streams(tab, features, rec_e, profile, perm, cfg=_DEFAULT_CFG):
    """Per-core bf16 feature stream (pre-scaled by 1/degree) + vin stream.

    Row layout is (q f): word q*F + f, so the PSUM pair halves are the
    contiguous column blocks [0:F] and [F:2F]."""
    prof = np.asarray(profile, dtype=np.int64)
    TT = int(prof.sum())
    wpair = (prof[0::2] + prof[1::2]) * ROW_W
    peb = np.concatenate([[0], np.cumsum(BLK * wpair)]).astype(np.int64)
    tb = np.concatenate([[0], np.cumsum(prof)]).astype(np.int64)
    TW = int(peb[-1])
    soff = np.zeros(NBLK, dtype=np.int64)
    soff[1::2] = prof[0::2] * ROW_W

    inv = np.empty((NCORES, NBLK), dtype=np.int64)
    for c in range(NCORES):
        inv[c, perm[c]] = np.arange(NBLK)

    scaled = features[tab["order"]] * rec_e[:, None]
    hi_u = scaled.astype(ml_dtypes.bfloat16).view(np.uint16)

    core = tab["core"]
    s_e = inv[core, tab["blk"]]
    p = tab["pr"] & (BLK - 1)
    i = tab["pr"] >> 7
    q = tab["q"]
    pid_e = s_e >> 1
    pos = (
        core * TW + peb[pid_e] + p * wpair[pid_e] + soff[s_e] + i * ROW_W + q * F
    )
    stream = np.zeros(NCORES * TW, dtype=np.uint16)
    cols = np.arange(F, dtype=np.int64)[None, :]
    stream[pos[:, None] + cols] = hi_u
    stream = stream.reshape(NCORES, TW).view(ml_dtypes.bfloat16)

    # vin stream [NCORES, 128, TT]; padding rows get -1 (never matches iota)
    vin_arr = np.full(NCORES * BLK * TT, -1.0, dtype=ml_dtypes.bfloat16)
    m0 = q == 0
    flat = core[m0] * (BLK * TT) + p[m0] * TT + (tb[s_e[m0]] + i[m0])
    vin_arr[flat] = tab["vin"][m0].astype(ml_dtypes.bfloat16)
    vin_arr = vin_arr.reshape(NCORES, BLK, TT)
    return stream, vin_arr


def edge_recip(adjacency, tab, t):
    """1/degree at each sorted edge's destination for table t."""
    adj = np.asarray(adjacency).reshape(V, T, N)
    deg = np.maximum((adj[:, t] >= 0).sum(axis=-1), 1).astype(np.float64)  # [V]
    rec = (1.0 / deg).astype(np.float32)
    return rec[tab["vglob"]]


def prepare_inputs(adjacency, indices0, features0, indices1, features1, cfg=_DEFAULT_CFG, oh_chunk=8):
    tab0 = shard_table(np.asarray(indices0), cfg=cfg)
    tab1 = shard_table(np.asarray(indices1), cfg=cfg)
    profile, meta = make_profile([tab0, tab1])

    r0 = edge_recip(adjacency, tab0, 0)
    r1 = edge_recip(adjacency, tab1, 1)
    f0, v0 = fill_streams(tab0, np.asarray(features0, dtype=np.float32), r0, profile, meta["perm"][0], cfg)
    f1, v1 = fill_streams(tab1, np.asarray(features1, dtype=np.float32), r1, profile, meta["perm"][1], cfg)
    iota = np.broadcast_to(
        np.arange(BLK_V).astype(ml_dtypes.bfloat16), (BLK, BLK_V)
    ).copy()
    iotar = np.broadcast_to(
        (np.arange(oh_chunk * BLK_V) // oh_chunk).astype(ml_dtypes.bfloat16),
        (BLK, oh_chunk * BLK_V),
    ).copy()

    in_maps = [
        {
            "feat0": f0[c],
            "feat1": f1[c],
            "vin0": v0[c],
            "vin1": v1[c],
            "iota": iota,
            "iotar": iotar,
        }
        for c in range(NCORES)
    ]
    return in_maps, profile, meta


def assemble_output(core_outs, meta, cfg=_DEFAULT_CFG):
    outs = []
    for t in range(T):
        perm = meta["perm"][t]
        vblk, vvin = meta["vmap"][t]
        parts = []
        for c in range(NCORES):
            arr = np.asarray(core_outs[c]).astype(np.float32)
            arr = arr.reshape(NGRP, BLK_V, G, T, F)[:, :, :, t, :]
            arr = arr.transpose(0, 2, 1, 3).reshape(NGRP * G, BLK_V, F)[:NBLK]
            tmp = np.empty((NBLK, BLK_V, F), dtype=np.float32)
            tmp[perm[c]] = arr
            parts.append(tmp[vblk[c, :VLOC], vvin[c, :VLOC]])
        outs.append(np.concatenate(parts, axis=0).reshape(B, V, F))
    return (outs[0], outs[1])


def kernel(adjacency, indices0, features0, indices1, features1):
    from concourse.bass_utils import run_bass_kernel_spmd

    cfg = _DEFAULT_CFG
    in_maps, profile, meta = prepare_inputs(
        adjacency, indices0, features0, indices1, features1, cfg
    )

    key = tuple(profile)
    if key not in _NC_CACHE:
        _NC_CACHE[key] = build_device_program(profile, cfg)
    nc = _NC_CACHE[key]

    res = run_bass_kernel_spmd(nc, in_maps, list(range(NCORES)))
    return assemble_output(
        [res.results[c]["out"] for c in range(NCORES)], meta, cfg
    )
